# revision 1
# baseline (speedup 1.0000x reference)
"""Trainium2 Bass kernel for nn_GatedLinearAttention (bidirectional GLA vision block).

Strategy
--------
Data-parallel over batch: 16 batch items -> 8 cores x 2 items. No collectives.

The chunked GLA scan is reformulated as *quadratic causal attention with global
decay* (mathematically exact):   o_t = sum_{s<=t} exp(B_t - B_s) (q_t . k_s) v_s
with B = running cumsum of log-gates, so qs = q*exp(B), ks = k*exp(-B) and the
whole scan becomes one masked matmul pair per (batch, head, direction).  The
backward direction is the same with a reverse cumsum and an anti-causal mask.
Decay totals are ~-34 in log space so exp(+-34) stays inside fp32/bf16 range.

Activations are feature-major [D, tokens] in SBUF; every projection is a
natural PE matmul and can produce outputs in either orientation.  v and the
attention output come out token-major, making per-token RMS scalars free.

ACT uses only {Sigmoid} and {Ln, Exp} LUT sets (plus universal Copy/Square):
silu(x) = x*sigmoid(x), log_sigmoid(u) = Ln(Sigmoid(u)),
rsqrt(m) = Exp(-0.5*Ln(m)).  Matmul inputs bf16, fp32 accumulation in PSUM.
"""

import os
import sys
from contextlib import ExitStack

for _p in ("/opt/trn_rl_repo", "/root/.axon_site/_ro/trn_rl_repo"):
    if os.path.isdir(_p) and _p not in sys.path:
        sys.path.insert(0, _p)

import numpy as np
import ml_dtypes

import concourse.bass as bass
import concourse.tile as tile
import concourse.mybir as mybir
from concourse.bass_utils import run_bass_kernel_spmd

f32 = mybir.dt.float32
bf16 = mybir.dt.bfloat16
AF = mybir.ActivationFunctionType
ALU = mybir.AluOpType

P = 128
NCORES = 8
B = 2               # batch items per core
L = 784             # tokens per batch item (28*28)
T = B * L           # tokens per core
D = 1024            # d_model
NH = 4
HDK = 256           # per-head key dim (2 partition tiles)
HDV = 512           # per-head value dim
GLN = 16.0
EPS = 1e-5
NT7 = 7             # batch-local token tiles (6*128 + 16)
TW = [128, 128, 128, 128, 128, 128, 16]
SW = TW
TC2 = [(0, 392), (392, 392)]              # batch-local 392-col chunks
ACH = [(0, 512), (512, 272)]              # batch-local A-phase t-chunks
DEBUG_OUT = bool(int(os.environ.get("GLA_DEBUG_OUT", "0")))


def _legalize_sync_waits(nc, max_waits=1):
    """The walrus shipped here rejects >1 semaphore wait per instruction.
    Split excess waits onto chained NOPs on the same engine right before the
    offending instruction: engines run their stream in order, so blocking
    earlier is equivalent."""
    counter = 0
    for fn in nc.m.functions:
        for blk in fn.blocks:
            insts = list(blk.instructions)
            changed = False
            out = []
            for inst in insts:
                si = inst.sync_info
                if si is not None and len(si.on_wait) > max_waits:
                    waits = list(si.on_wait)
                    keep = waits[len(waits) - max_waits:]
                    move = waits[: len(waits) - max_waits]
                    for i in range(0, len(move), max_waits):
                        chunk = move[i: i + max_waits]
                        nop = mybir.InstNoOp(
                            name=f"legalize-wait-nop-{counter}", ins=[], outs=[]
                        )
                        counter += 1
                        nop.engine = inst.engine
                        nop.sync_info = mybir.SyncInfo(on_wait=chunk, on_update=[])
                        out.append(nop)
                    inst.sync_info = mybir.SyncInfo(
                        on_wait=keep, on_update=list(si.on_update)
                    )
                    changed = True
                out.append(inst)
            if changed:
                blk.instructions = out


def _build_program():
    nc = bass.Bass()

    xpad_d = nc.dram_tensor("xpad", [8, P, B * 30 * 30], bf16, kind="ExternalInput")
    cdg_d = nc.dram_tensor("cdg", [9, 8, P, P], bf16, kind="ExternalInput")
    qkvw_d = nc.dram_tensor("qkvw", [8, P, 4096], bf16, kind="ExternalInput")
    gk1w_d = nc.dram_tensor("gk1w", [8, P, 16], bf16, kind="ExternalInput")
    gk2w_d = nc.dram_tensor("gk2w", [16, 2048], bf16, kind="ExternalInput")
    b2_d = nc.dram_tensor("b2", [16, P, 1], f32, kind="ExternalInput")
    gw_d = nc.dram_tensor("gw", [8, P, 2048], bf16, kind="ExternalInput")
    ow_d = nc.dram_tensor("ow", [16, P, 1024], bf16, kind="ExternalInput")
    masks_d = nc.dram_tensor("masks", [8, P, 512], bf16, kind="ExternalInput")
    out_d = nc.dram_tensor("out", [T, 1024], f32, kind="ExternalOutput")
    dbg = {}
    if DEBUG_OUT:
        dbg["xc"] = nc.dram_tensor("dbg_xc", [8, P, T], f32, kind="ExternalOutput")
        dbg["cs"] = nc.dram_tensor("dbg_cs", [4, P, L], f32, kind="ExternalOutput")
        dbg["qsf"] = nc.dram_tensor("dbg_qsf", [2, P, L], f32, kind="ExternalOutput")
        dbg["am"] = nc.dram_tensor("dbg_am", [P, NT7 * L], f32, kind="ExternalOutput")
        dbg["ofr"] = nc.dram_tensor("dbg_ofr", [P, NT7 * HDV], f32, kind="ExternalOutput")

    with tile.TileContext(nc) as tc:
        with ExitStack() as ctx:
            cst = ctx.enter_context(tc.tile_pool(name="cst", bufs=1))
            big = ctx.enter_context(tc.tile_pool(name="big", bufs=1))
            wts = ctx.enter_context(tc.tile_pool(name="wts", bufs=1))
            gat = ctx.enter_context(tc.tile_pool(name="gat", bufs=1))
            mid = ctx.enter_context(tc.tile_pool(name="mid", bufs=1))
            sm1 = ctx.enter_context(tc.tile_pool(name="sm1", bufs=1))
            sm2 = ctx.enter_context(tc.tile_pool(name="sm2", bufs=2))
            ps = ctx.enter_context(tc.tile_pool(name="ps", bufs=8, space="PSUM"))

            def psum(rows, cols):
                pstile = ps.tile([P, 512], f32, tag="ps", name="pstile")
                return pstile[:rows, :cols]

            # ---- constants ----
            masks = cst.tile([P, 8, 512], bf16)
            nc.sync.dma_start(out=masks, in_=masks_d.rearrange("m p t -> p m t"))
            zeros = cst.tile([P, L], f32)
            nc.vector.memset(zeros[:], 0.0)
            epst = cst.tile([P, 1], f32)
            nc.vector.memset(epst[:], EPS)

            # ---- persistent activations ----
            xc = big.tile([P, 8, T], bf16)           # conv+silu output, feature-major
            gk1o = big.tile([16, T], bf16)           # low-rank gate bottleneck
            og = big.tile([P, NT7, 2048], bf16)      # gated attn out (one batch), token-major

            # ================= Stage A: depthwise conv 3x3 + silu =================
            for ft in range(8):
                xp = gat.tile([P, B, 30, 30], bf16, tag="xp")
                nc.sync.dma_start(out=xp, in_=xpad_d[ft].rearrange("p (b h w) -> p b h w", b=B, h=30))
                cd = gat.tile([P, 9, P], bf16, tag="cd")
                nc.sync.dma_start(out=cd, in_=cdg_d[:, ft].rearrange("m p q -> p m q"))
                for bi in range(B):
                    for half in range(2):
                        pt = psum(P, 392)
                        for tap in range(9):
                            a, bb = tap // 3, tap % 3
                            rhs = xp[:, bi, a + half * 14: a + half * 14 + 14, bb: bb + 28]
                            nc.tensor.matmul(pt, cd[:, tap, :], rhs,
                                             start=(tap == 0), stop=(tap == 8))
                        sgc = sm2.tile([P, 392], f32, tag="sgc")
                        nc.scalar.activation(sgc, pt, AF.Sigmoid)
                        dst = xc[:, ft, bi * L + half * 392: bi * L + (half + 1) * 392]
                        nc.vector.tensor_mul(dst, pt, sgc)
                if DEBUG_OUT:
                    xcf = sm2.tile([P, T], f32, tag="dbgxc")
                    nc.vector.tensor_copy(xcf, xc[:, ft, :])
                    nc.sync.dma_start(out=dbg["xc"][ft], in_=xcf)

            # ================= Stage B: gk1 bottleneck [16, T] =================
            w1 = wts.tile([P, 8, 16], bf16, tag="w1")
            nc.sync.dma_start(out=w1, in_=gk1w_d.rearrange("k p c -> p k c"))
            for tc4 in range(4):
                pt = psum(16, 392)
                for kt in range(8):
                    nc.tensor.matmul(pt, w1[:, kt, :], xc[:, kt, tc4 * 392:(tc4 + 1) * 392],
                                     start=(kt == 0), stop=(kt == 7))
                nc.scalar.copy(gk1o[:, tc4 * 392:(tc4 + 1) * 392], pt)

            # ================= per (batch, head) =================
            for bi in range(B):
                for h in range(NH):
                    # ---- weights for this head ----
                    wq = gat.tile([P, 8, HDK], bf16, tag="wq")
                    nc.sync.dma_start(out=wq, in_=qkvw_d[:, :, h * HDK:(h + 1) * HDK].rearrange("k p c -> p k c"))
                    wk = gat.tile([P, 8, HDK], bf16, tag="wk")
                    nc.sync.dma_start(out=wk, in_=qkvw_d[:, :, 1024 + h * HDK: 1024 + (h + 1) * HDK].rearrange("k p c -> p k c"))
                    wv = gat.tile([P, 8, HDV], bf16, tag="wv")
                    nc.sync.dma_start(out=wv, in_=qkvw_d[:, :, 2048 + h * HDV: 2048 + (h + 1) * HDV].rearrange("k p c -> p k c"))
                    gwt = gat.tile([P, 8, HDV], bf16, tag="gw")
                    nc.sync.dma_start(out=gwt, in_=gw_d[:, :, h * HDV:(h + 1) * HDV].rearrange("k p c -> p k c"))
                    w2 = gat.tile([16, 4, P], bf16, tag="w2")
                    nc.sync.dma_start(out=w2[:, 0:2, :], in_=gk2w_d[:, h * HDK:(h + 1) * HDK].rearrange("k (c p) -> k c p", c=2))
                    nc.sync.dma_start(out=w2[:, 2:4, :], in_=gk2w_d[:, 1024 + h * HDK: 1024 + (h + 1) * HDK].rearrange("k (c p) -> k c p", c=2))
                    b2t = gat.tile([P, 4], f32, tag="b2")
                    for mi, mt in enumerate([2 * h, 2 * h + 1, 8 + 2 * h, 8 + 2 * h + 1]):
                        nc.sync.dma_start(out=b2t[:, mi: mi + 1], in_=b2_d[mt])

                    # ---- gate slab for this head: silu(xc @ g_w) token-major ----
                    gate_h = mid.tile([P, NT7, HDV], bf16, tag="gate")
                    for tt in range(NT7):
                        tw = TW[tt]
                        pt = psum(tw, HDV)
                        for kt in range(8):
                            nc.tensor.matmul(pt, xc[:, kt, bi * L + tt * P: bi * L + tt * P + tw],
                                             gwt[:, kt, :], start=(kt == 0), stop=(kt == 7))
                        gsc = sm2.tile([P, HDV], f32, tag="gsig")
                        nc.scalar.activation(gsc[:tw], pt, AF.Sigmoid)
                        nc.vector.tensor_mul(gate_h[:tw, tt, :], pt, gsc[:tw])

                    # ---- v projection (token-major) ----
                    vh = mid.tile([P, NT7, HDV], bf16, tag="vh")
                    for tt in range(NT7):
                        tw = TW[tt]
                        pt = psum(tw, HDV)
                        for kt in range(8):
                            nc.tensor.matmul(pt, xc[:, kt, bi * L + tt * P: bi * L + tt * P + tw],
                                             wv[:, kt, :], start=(kt == 0), stop=(kt == 7))
                        nc.scalar.copy(vh[:tw, tt, :], pt)

                    # ---- decays + q,k projections, per column-tile ct ----
                    qsf = mid.tile([P, 2, L], bf16, tag="qsf")
                    qsb = mid.tile([P, 2, L], bf16, tag="qsb")
                    ksf = mid.tile([P, 2, L], bf16, tag="ksf")
                    ksb = mid.tile([P, 2, L], bf16, tag="ksb")
                    for ct in range(2):
                        ets = []
                        for dr in range(2):
                            mi = dr * 2 + ct
                            t1 = sm1.tile([P, L], f32, tag="t1")
                            for tc_ in range(2):
                                o0, w0 = TC2[tc_]
                                pt = psum(P, 392)
                                nc.tensor.matmul(pt, w2[:, mi, :],
                                                 gk1o[:, bi * L + o0: bi * L + o0 + w0],
                                                 start=True, stop=True)
                                nc.scalar.activation(t1[:, o0:o0 + w0], pt, AF.Sigmoid,
                                                     bias=b2t[:, mi: mi + 1])
                            t2 = sm1.tile([P, L], f32, tag="t2")
                            nc.scalar.activation(t2, t1, AF.Ln)     # log_sigmoid(u)
                            nc.vector.tensor_tensor_scan(t1, t2, zeros, 0.0, ALU.add, ALU.add)
                            src = t1                                 # cs = cumsum(ls)
                            if dr == 1:
                                # reverse-inclusive cumsum: csr = ls - cs + total
                                nc.vector.tensor_sub(t2, t2, t1)
                                nc.vector.tensor_scalar_add(t2, t2, t1[:, L - 1: L])
                                src = t2
                            eq = sm1.tile([P, L], bf16, tag=f"eq{dr}")
                            nc.scalar.activation(eq, src, AF.Exp, scale=1.0 / GLN)
                            ek = sm1.tile([P, L], bf16, tag=f"ek{dr}")
                            nc.scalar.activation(ek, src, AF.Exp, scale=-1.0 / GLN)
                            ets.append((eq, ek))
                            if DEBUG_OUT and bi == 0 and h == 0:
                                csf = sm2.tile([P, L], f32, tag="dbgcs")
                                nc.vector.tensor_copy(csf, src)
                                nc.sync.dma_start(out=dbg["cs"][mi], in_=csf)
                        for tc_ in range(2):
                            o0, w0 = TC2[tc_]
                            sl = slice(o0, o0 + w0)
                            pt = psum(P, 392)
                            for kt in range(8):
                                nc.tensor.matmul(pt, wq[:, kt, ct * P:(ct + 1) * P],
                                                 xc[:, kt, bi * L + o0: bi * L + o0 + w0],
                                                 start=(kt == 0), stop=(kt == 7))
                            nc.vector.tensor_mul(qsf[:, ct, sl], pt, ets[0][0][:, sl])
                            nc.vector.tensor_mul(qsb[:, ct, sl], pt, ets[1][0][:, sl])
                            pt = psum(P, 392)
                            for kt in range(8):
                                nc.tensor.matmul(pt, wk[:, kt, ct * P:(ct + 1) * P],
                                                 xc[:, kt, bi * L + o0: bi * L + o0 + w0],
                                                 start=(kt == 0), stop=(kt == 7))
                            nc.vector.tensor_mul(ksf[:, ct, sl], pt, ets[0][1][:, sl])
                            nc.vector.tensor_mul(ksb[:, ct, sl], pt, ets[1][1][:, sl])
                    if DEBUG_OUT and bi == 0 and h == 0:
                        for ct in range(2):
                            qf = sm2.tile([P, L], f32, tag="dbgqs")
                            nc.vector.tensor_copy(qf, qsf[:, ct, :])
                            nc.sync.dma_start(out=dbg["qsf"][ct], in_=qf)

                    # ---- A + o per direction ----
                    ofn = None
                    for dr in range(2):
                        qs = qsf if dr == 0 else qsb
                        ks = ksf if dr == 0 else ksb
                        am = mid.tile([P, NT7, L], bf16, tag="am")
                        for j in range(2):
                            jo, jw = ACH[j]
                            for si in range(NT7):
                                d = si - 4 * j
                                if dr == 0:
                                    if si * P > jo + jw - 1:
                                        continue        # fully masked
                                    mi_ = None if d < 0 else d
                                else:
                                    if si * P + SW[si] - 1 < jo:
                                        continue
                                    mi_ = None if d >= 4 else 4 + d
                                sw = SW[si]
                                pt = psum(sw, jw)
                                for ct in range(2):
                                    nc.tensor.matmul(pt, ks[:, ct, si * P: si * P + sw],
                                                     qs[:, ct, jo: jo + jw],
                                                     start=(ct == 0), stop=(ct == 1))
                                if mi_ is None:
                                    nc.scalar.copy(am[:sw, si, jo: jo + jw], pt)
                                else:
                                    nc.vector.tensor_mul(am[:sw, si, jo: jo + jw], pt,
                                                         masks[:sw, mi_, :jw])
                        if DEBUG_OUT and bi == 0 and h == 0 and dr == 0:
                            amf = sm2.tile([P, NT7 * L], f32, tag="dbgam")
                            nc.vector.tensor_copy(amf, am.rearrange("p a b -> p (a b)"))
                            nc.sync.dma_start(out=dbg["am"], in_=amf)

                        ofr = mid.tile([P, NT7, HDV], bf16, tag=f"ofr{dr}")
                        ssq = sm1.tile([P, 8], f32, tag="ssq")
                        nc.vector.memset(ssq[:], 0.0)
                        scrap = sm1.tile([P, HDV], bf16, tag="scrap")
                        for tt in range(NT7):
                            tw = TW[tt]
                            sis = list(range(0, tt + 1) if dr == 0 else range(tt, NT7))
                            pt = psum(tw, HDV)
                            for ii, si in enumerate(sis):
                                nc.tensor.matmul(pt, am[:SW[si], si, tt * P: tt * P + tw],
                                                 vh[:SW[si], si, :],
                                                 start=(ii == 0), stop=(ii == len(sis) - 1))
                            nc.scalar.activation(scrap[:tw], pt, AF.Square,
                                                 accum_out=ssq[:tw, tt: tt + 1])
                            nc.scalar.copy(ofr[:tw, tt, :], pt)
                        # r = (ssq/512 + eps)^-1/2 = exp(-0.5 * ln(ssq/512 + eps))
                        rsl = sm1.tile([P, 8], f32, tag="rsl")
                        nc.scalar.activation(rsl, ssq, AF.Ln, scale=1.0 / HDV, bias=epst[:])
                        nc.scalar.activation(rsl, rsl, AF.Exp, scale=-0.5)
                        if dr == 0:
                            for tt in range(NT7):
                                nc.vector.tensor_scalar_mul(ofr[:TW[tt], tt, :], ofr[:TW[tt], tt, :],
                                                            rsl[:TW[tt], tt: tt + 1])
                            ofn = ofr
                            if DEBUG_OUT and bi == 0 and h == 0:
                                off = sm2.tile([P, NT7 * HDV], f32, tag="dbgof")
                                nc.vector.tensor_copy(off, ofr.rearrange("p a b -> p (a b)"))
                                nc.sync.dma_start(out=dbg["ofr"], in_=off)
                        else:
                            for tt in range(NT7):
                                tw = TW[tt]
                                nc.vector.scalar_tensor_tensor(
                                    ofr[:tw, tt, :], ofr[:tw, tt, :], rsl[:tw, tt: tt + 1],
                                    ofn[:tw, tt, :], ALU.mult, ALU.add)
                                nc.vector.tensor_mul(og[:tw, tt, h * HDV:(h + 1) * HDV],
                                                     ofr[:tw, tt, :],
                                                     gate_h[:tw, tt, :])

                # ======== Stage E for this batch: out = og @ o_w ========
                for nch in range(2):
                    owh = wts.tile([P, 16, 512], bf16, tag="owh")
                    nc.sync.dma_start(out=owh, in_=ow_d[:, :, nch * 512:(nch + 1) * 512].rearrange("j p c -> p j c"))
                    for g0 in range(0, NT7, 2):
                        tts = [tt for tt in (g0, g0 + 1) if tt < NT7]
                        ogT = gat.tile([P, 2, 16, P], bf16, tag="ogT")
                        for i, tt in enumerate(tts):
                            for jt in range(16):
                                nc.sync.dma_start_transpose(ogT[:, i, jt, :TW[tt]],
                                                            og[:TW[tt], tt, jt * P:(jt + 1) * P])
                        pts = [psum(TW[tt], 512) for tt in tts]
                        for jt in range(16):
                            for i, tt in enumerate(tts):
                                nc.tensor.matmul(pts[i], ogT[:, i, jt, :TW[tt]],
                                                 owh[:, jt, :],
                                                 start=(jt == 0), stop=(jt == 15))
                        for i, tt in enumerate(tts):
                            outs = sm2.tile([P, 512], f32, tag="outs")
                            nc.scalar.copy(outs[:TW[tt], :], pts[i])
                            nc.sync.dma_start(
                                out=out_d[bi * L + tt * P: bi * L + tt * P + TW[tt],
                                          nch * 512:(nch + 1) * 512],
                                in_=outs[:TW[tt], :])

    _legalize_sync_waits(nc)
    return nc


_CACHE = {}


def _prep_shared(conv_w, qkv_w, gk_w1, gk_w2, gk_b2, g_w, o_w, gnorm_w, lnorm_w):
    bf = ml_dtypes.bfloat16
    cdg = np.zeros((9, 8, P, P), np.float32)
    w9 = conv_w.reshape(9, D)  # taps x channels (HWIO with I=1)
    idx = np.arange(P)
    for tap in range(9):
        for ft in range(8):
            cdg[tap, ft, idx, idx] = w9[tap, ft * P:(ft + 1) * P]
    assert np.allclose(gnorm_w, lnorm_w), "kernel assumes gnorm_w == lnorm_w (fold into o_w)"
    ow_eff = o_w * np.tile(gnorm_w, NH)[:, None]
    masks = np.zeros((8, P, 512), np.float32)
    s_i = np.arange(P)[:, None]
    t_i = np.arange(512)[None, :]
    for dd in range(4):
        masks[dd] = (s_i <= t_i - P * dd)
        masks[4 + dd] = (s_i >= t_i - P * dd)
    return {
        "cdg": np.ascontiguousarray(cdg.astype(bf)),
        "qkvw": np.ascontiguousarray(qkv_w.reshape(8, P, 4096).astype(bf)),
        "gk1w": np.ascontiguousarray(gk_w1.reshape(8, P, 16).astype(bf)),
        "gk2w": np.ascontiguousarray(gk_w2.astype(bf)),
        "b2": np.ascontiguousarray(gk_b2.reshape(16, P, 1).astype(np.float32)),
        "gw": np.ascontiguousarray(g_w.reshape(8, P, 2048).astype(bf)),
        "ow": np.ascontiguousarray(ow_eff.reshape(16, P, 1024).astype(bf)),
        "masks": np.ascontiguousarray(masks.astype(bf)),
    }


def kernel(x, conv_w, qkv_w, gk_w1, gk_w2, gk_b2, g_w, g_b, o_w, gnorm_w, lnorm_w, H, W,
           _return_res=False, _trace=False):
    x = np.asarray(x, np.float32)
    assert int(H) == 28 and int(W) == 28 and x.shape == (16, L, D)
    assert np.allclose(np.asarray(g_b), 0.0), "kernel assumes g_b == 0"
    bf = ml_dtypes.bfloat16

    if "nc" not in _CACHE:
        _CACHE["nc"] = _build_program()
    nc = _CACHE["nc"]

    shared = _prep_shared(np.asarray(conv_w, np.float32), np.asarray(qkv_w, np.float32),
                          np.asarray(gk_w1, np.float32), np.asarray(gk_w2, np.float32),
                          np.asarray(gk_b2, np.float32), np.asarray(g_w, np.float32),
                          np.asarray(o_w, np.float32), np.asarray(gnorm_w, np.float32),
                          np.asarray(lnorm_w, np.float32))
    in_maps = []
    for c in range(NCORES):
        xs = x[2 * c: 2 * c + 2]                       # [2, 784, 1024]
        xt = xs.reshape(B, 28, 28, D).transpose(3, 0, 1, 2)   # [1024, 2, 28, 28]
        xpad = np.zeros((D, B, 30, 30), np.float32)
        xpad[:, :, 1:29, 1:29] = xt
        m = dict(shared)
        m["xpad"] = np.ascontiguousarray(xpad.reshape(8, P, B * 900).astype(bf))
        in_maps.append(m)

    res = run_bass_kernel_spmd(nc, in_maps, core_ids=list(range(NCORES)), trace=_trace)
    out = np.concatenate([r["out"].reshape(B, L, D) for r in res.results], axis=0)
    if _return_res:
        return out, res
    return out



# revision 2
# speedup vs baseline: 1.0138x; 1.0138x over previous
"""Trainium2 Bass kernel for nn_GatedLinearAttention (bidirectional GLA vision block), v2.

Same math as baseline (quadratic masked attention with global decay), restructured
for PE continuity:
  - one activation-table epoch switch total: Silu (conv) -> natural_log_exp for the
    rest: log_sigmoid(u) = -ln(1 + exp(-u)) (Exp then Ln with bias=1), gates via
    silu(g) = g / (1 + exp(-g)) (scalar Exp + DVE reciprocal), rsqrt via Ln+Exp.
  - decay chains computed one combo ahead (scalar/gpsimd/DVE work hidden under PE).
  - cumsum scans + reverse fixes moved to the idle GpSimd engine.
  - og transposed via PE identity-matmul transposes (not serialized DMA transposes).
  - out projection reads transposed og tiles; weights streamed per half-slab.
Data-parallel over batch: 16 batch items -> 8 cores x 2. No collectives.
"""

import os
import sys
from contextlib import ExitStack

for _p in ("/opt/trn_rl_repo", "/root/.axon_site/_ro/trn_rl_repo"):
    if os.path.isdir(_p) and _p not in sys.path:
        sys.path.insert(0, _p)

import numpy as np
import ml_dtypes

import concourse.bass as bass
import concourse.tile as tile
import concourse.mybir as mybir
from concourse.bass_utils import run_bass_kernel_spmd

f32 = mybir.dt.float32
bf16 = mybir.dt.bfloat16
AF = mybir.ActivationFunctionType
ALU = mybir.AluOpType

P = 128
NCORES = 8
B = 2               # batch items per core
L = 784             # tokens per batch item
T = B * L
D = 1024
NH = 4
HDK = 256
HDV = 512
GLN = 16.0
EPS = 1e-5
NT7 = 7
TW = [128, 128, 128, 128, 128, 128, 16]
SW = TW
TC2 = [(0, 392), (392, 392)]
ACH = [(0, 512), (512, 272)]


def _legalize_sync_waits(nc, max_waits=1):
    """Split >1 semaphore waits onto chained NOPs (walrus limitation)."""
    counter = 0
    for fn in nc.m.functions:
        for blk in fn.blocks:
            insts = list(blk.instructions)
            changed = False
            out = []
            for inst in insts:
                si = inst.sync_info
                if si is not None and len(si.on_wait) > max_waits:
                    waits = list(si.on_wait)
                    keep = waits[len(waits) - max_waits:]
                    move = waits[: len(waits) - max_waits]
                    for i in range(0, len(move), max_waits):
                        chunk = move[i: i + max_waits]
                        nop = mybir.InstNoOp(
                            name=f"legalize-wait-nop-{counter}", ins=[], outs=[]
                        )
                        counter += 1
                        nop.engine = inst.engine
                        nop.sync_info = mybir.SyncInfo(on_wait=chunk, on_update=[])
                        out.append(nop)
                    inst.sync_info = mybir.SyncInfo(
                        on_wait=keep, on_update=list(si.on_update)
                    )
                    changed = True
                out.append(inst)
            if changed:
                blk.instructions = out


def _build_program():
    nc = bass.Bass()

    xpad_d = nc.dram_tensor("xpad", [8, P, B * 30 * 30], bf16, kind="ExternalInput")
    cw_d = nc.dram_tensor("cw", [8, P, 9], f32, kind="ExternalInput")
    wcat_d = nc.dram_tensor("wcat", [NH, 8, P, 1024], bf16, kind="ExternalInput")
    gwc_d = nc.dram_tensor("gwc", [NH, 8, P, 512], bf16, kind="ExternalInput")
    gk1w_d = nc.dram_tensor("gk1w", [8, P, 16], bf16, kind="ExternalInput")
    gk2w_d = nc.dram_tensor("gk2w", [16, 2048], bf16, kind="ExternalInput")
    b2n_d = nc.dram_tensor("b2n", [16, P, 1], f32, kind="ExternalInput")
    ow_d = nc.dram_tensor("ow", [16, P, 1024], bf16, kind="ExternalInput")
    f8 = mybir.dt.float8e4
    masks_d = nc.dram_tensor("masks", [8, P, 512], f8, kind="ExternalInput")
    ident_d = nc.dram_tensor("ident", [P, P], bf16, kind="ExternalInput")
    out_d = nc.dram_tensor("out", [T, 1024], f32, kind="ExternalOutput")

    with tile.TileContext(nc) as tc:
        with ExitStack() as ctx:
            cst = ctx.enter_context(tc.tile_pool(name="cst", bufs=1))
            big = ctx.enter_context(tc.tile_pool(name="big", bufs=1))
            wc = ctx.enter_context(tc.tile_pool(name="wc", bufs=2))
            gwp = ctx.enter_context(tc.tile_pool(name="gwp", bufs=1))
            bsl = ctx.enter_context(tc.tile_pool(name="bsl", bufs=2))
            dec = ctx.enter_context(tc.tile_pool(name="dec", bufs=2))
            cpp = ctx.enter_context(tc.tile_pool(name="cpp", bufs=1))
            qk = ctx.enter_context(tc.tile_pool(name="qk", bufs=1))
            mid = ctx.enter_context(tc.tile_pool(name="mid", bufs=1))
            ogp = ctx.enter_context(tc.tile_pool(name="ogp", bufs=1))
            ogTp = ctx.enter_context(tc.tile_pool(name="ogTp", bufs=4))
            sout = ctx.enter_context(tc.tile_pool(name="sout", bufs=2))
            outp = ctx.enter_context(tc.tile_pool(name="outp", bufs=2))
            xpp = ctx.enter_context(tc.tile_pool(name="xpp", bufs=2))
            cdp = ctx.enter_context(tc.tile_pool(name="cdp", bufs=1))
            ps = ctx.enter_context(tc.tile_pool(name="ps", bufs=6, space="PSUM"))
            tps = ctx.enter_context(tc.tile_pool(name="tps", bufs=2, space="PSUM"))

            def psum(rows, cols):
                pstile = ps.tile([P, 512], f32, tag="ps", name="pstile")
                return pstile[:rows, :cols]

            # ---- constants ----
            wvec = cst.tile([P, 8, 9], f32)
            nc.gpsimd.dma_start(out=wvec, in_=cw_d.rearrange("f p t -> p f t"))
            ident = cst.tile([P, P], bf16)
            nc.gpsimd.dma_start(out=ident, in_=ident_d[:])
            masks = cst.tile([P, 8, 512], f8)
            nc.gpsimd.dma_start(out=masks, in_=masks_d.rearrange("m p t -> p m t"))
            zeros = cst.tile([P, 392], bf16)
            nc.vector.memset(zeros[:], 0.0)
            epsone = cst.tile([P, 2], f32)
            nc.vector.memset(epsone[:, 0:1], EPS)
            nc.vector.memset(epsone[:, 1:2], 1.0)
            epst = epsone[:, 0:1]
            onet = epsone[:, 1:2]
            w1 = cst.tile([P, 8, 16], bf16)
            nc.gpsimd.dma_start(out=w1, in_=gk1w_d.rearrange("k p c -> p k c"))

            # ---- persistent activations ----
            xc = big.tile([P, 8, T], bf16)
            gk1o = big.tile([16, T], bf16)

            # ==== Stage A+B per batch item: conv 3x3 + silu (PE diag), gk1 ====
            def stage_ab(bi):
                for ft in range(8):
                    xp = xpp.tile([P, 30, 30], bf16, tag="xp", name="xp")
                    nc.gpsimd.dma_start(
                        out=xp, in_=xpad_d[ft].rearrange("p (b h w) -> p b h w", b=B, h=30)[:, bi])
                    cd = cdp.tile([P, 9, P], bf16, tag="cd", name="cd")
                    for tap in range(9):
                        nc.vector.tensor_scalar_mul(cd[:, tap, :], ident[:],
                                                    wvec[:, ft, tap: tap + 1])
                    pts = [psum(P, 392) for _ in range(2)]
                    for tap in range(9):
                        a, bb = tap // 3, tap % 3
                        for half in range(2):
                            rhs = xp[:, a + half * 14: a + half * 14 + 14, bb: bb + 28]
                            nc.tensor.matmul(pts[half], cd[:, tap, :], rhs,
                                             start=(tap == 0), stop=(tap == 8))
                    for half in range(2):
                        dst = xc[:, ft, bi * L + half * 392: bi * L + (half + 1) * 392]
                        nc.scalar.activation(dst, pts[half], AF.Silu)
                for tc2 in range(2):
                    pt = psum(16, 392)
                    for kt in range(8):
                        nc.tensor.matmul(pt, w1[:, kt, :],
                                         xc[:, kt, bi * L + tc2 * 392: bi * L + (tc2 + 1) * 392],
                                         start=(kt == 0), stop=(kt == 7))
                    nc.vector.tensor_copy(gk1o[:, bi * L + tc2 * 392: bi * L + (tc2 + 1) * 392], pt)

            # ============== decay chains (one combo of lookahead) ==============
            def emit_decay(c):
                bi, h = divmod(c, 4)
                w2 = cpp.tile([16, 4, P], bf16, tag="w2", name="w2")
                nc.gpsimd.dma_start(out=w2[:, 0:2, :],
                                  in_=gk2w_d[:, h * HDK:(h + 1) * HDK].rearrange("k (c p) -> k c p", c=2))
                nc.gpsimd.dma_start(out=w2[:, 2:4, :],
                                  in_=gk2w_d[:, 1024 + h * HDK: 1024 + (h + 1) * HDK].rearrange("k (c p) -> k c p", c=2))
                b2t = dec.tile([P, 4], f32, tag="b2t", name="b2t")
                for mi, mt in enumerate([2 * h, 2 * h + 1, 8 + 2 * h, 8 + 2 * h + 1]):
                    nc.gpsimd.dma_start(out=b2t[:, mi: mi + 1], in_=b2n_d[mt])
                eqf = dec.tile([P, 2, L], bf16, tag="eqf", name="eqf", bufs=1)
                eqb = dec.tile([P, 2, L], bf16, tag="eqb", name="eqb", bufs=1)
                ekf = dec.tile([P, 2, L], bf16, tag="ekf", name="ekf", bufs=1)
                ekb = dec.tile([P, 2, L], bf16, tag="ekb", name="ekb", bufs=1)
                for dr in range(2):
                    et = dec.tile([P, 2, L], bf16, tag="edec", name="et")
                    lp = dec.tile([P, 2, L], bf16, tag="edec", name="lp")
                    cp = cpp.tile([P, 2, L], f32, tag="cp", name="cp")
                    for ct in range(2):
                        mi = dr * 2 + ct
                        for o0, w0 in TC2:
                            upt = psum(P, 392)
                            nc.tensor.matmul(upt, w2[:, mi, :],
                                             gk1o[:, bi * L + o0: bi * L + o0 + w0],
                                             start=True, stop=True)
                            # e = exp(-(u0 + b2)) = exp(-u0 + b2neg)
                            nc.scalar.activation(et[:, ct, o0:o0 + w0], upt, AF.Exp,
                                                 scale=-1.0, bias=b2t[:, mi: mi + 1])
                        # lp = ln(1 + e) = softplus(-u) = -log_sigmoid(u)
                        nc.scalar.activation(lp[:, ct, :], et[:, ct, :], AF.Ln, bias=onet)
                        nc.vector.tensor_tensor_scan(cp[:, ct, 0:392], lp[:, ct, 0:392],
                                                     zeros[:], 0.0, ALU.add, ALU.add)
                        nc.vector.tensor_tensor_scan(cp[:, ct, 392:L], lp[:, ct, 392:L],
                                                     zeros[:], cp[:, ct, 391:392],
                                                     ALU.add, ALU.add)
                        if dr == 1:
                            # cpr = lp - cp + cp_total (reverse-inclusive cumsum)
                            tot = dec.tile([P, 1], f32, tag="tot", name="tot")
                            nc.vector.tensor_copy(tot[:, 0:1], cp[:, ct, L - 1:L])
                            nc.vector.tensor_sub(cp[:, ct, :], lp[:, ct, :], cp[:, ct, :])
                            nc.vector.tensor_scalar_add(cp[:, ct, :], cp[:, ct, :], tot[:, 0:1])
                    dq = eqf if dr == 0 else eqb
                    dk = ekf if dr == 0 else ekb
                    nc.scalar.activation(dq.rearrange("p a b -> p (a b)"),
                                         cp.rearrange("p a b -> p (a b)"),
                                         AF.Exp, scale=-1.0 / GLN)
                    nc.scalar.activation(dk.rearrange("p a b -> p (a b)"),
                                         cp.rearrange("p a b -> p (a b)"),
                                         AF.Exp, scale=1.0 / GLN)
                return eqf, eqb, ekf, ekb

            # prefetch weights for combo 0
            def load_wcat(h):
                w = wc.tile([P, 8, 1024], bf16, tag="wcat", name="wcat")
                nc.gpsimd.dma_start(out=w, in_=wcat_d[h].rearrange("k p c -> p k c"))
                return w

            def load_gw(h):
                g = gwp.tile([P, 8, 512], bf16, tag="gw", name="gw")
                nc.gpsimd.dma_start(out=g, in_=gwc_d[h].rearrange("k p c -> p k c"))
                return g

            stage_ab(0)
            wcats = {0: load_wcat(0)}
            gws = {0: load_gw(0)}
            decays = {0: emit_decay(0)}
            stage_ab(1)
            ogTs = {}

            for c in range(8):
                bi, h = divmod(c, 4)
                eqf, eqb, ekf, ekb = decays.pop(c)
                wcat = wcats.pop(c)
                gw = gws.pop(c)
                if c + 1 < 8:
                    wcats[c + 1] = load_wcat((c + 1) % 4)
                    gws[c + 1] = load_gw((c + 1) % 4)
                owh0 = []
                if h == 3:
                    for hf in range(2):
                        owh = bsl.tile([P, 8, 512], bf16, tag="bslab", name="owh")
                        nc.gpsimd.dma_start(
                            out=owh,
                            in_=ow_d[hf * 8:(hf + 1) * 8, :, 0:512]
                            .rearrange("j p c -> p j c"))
                        owh0.append(owh)

                # ---- q/k projections + decay muls (feature-major [feat, tok]) ----
                qsf = qk.tile([P, 2, L], bf16, tag="qsf", name="qsf")
                qsb = qk.tile([P, 2, L], bf16, tag="qsb", name="qsb")
                ksf = qk.tile([P, 2, L], bf16, tag="ksf", name="ksf")
                ksb = qk.tile([P, 2, L], bf16, tag="ksb", name="ksb")
                for ct in range(2):
                    for o0, w0 in TC2:
                        sl = slice(o0, o0 + w0)
                        qpt = psum(P, 392)
                        for kt in range(8):
                            nc.tensor.matmul(qpt, wcat[:, kt, ct * P:(ct + 1) * P],
                                             xc[:, kt, bi * L + o0: bi * L + o0 + w0],
                                             start=(kt == 0), stop=(kt == 7))
                        nc.vector.tensor_mul(qsf[:, ct, sl], qpt, eqf[:, ct, sl])
                        nc.vector.tensor_mul(qsb[:, ct, sl], qpt, eqb[:, ct, sl])
                        kpt = psum(P, 392)
                        for kt in range(8):
                            nc.tensor.matmul(kpt, wcat[:, kt, 256 + ct * P: 256 + (ct + 1) * P],
                                             xc[:, kt, bi * L + o0: bi * L + o0 + w0],
                                             start=(kt == 0), stop=(kt == 7))
                        nc.vector.tensor_mul(ksf[:, ct, sl], kpt, ekf[:, ct, sl])
                        nc.vector.tensor_mul(ksb[:, ct, sl], kpt, ekb[:, ct, sl])

                if c + 1 < 8:
                    decays[c + 1] = emit_decay(c + 1)

                # ---- A phase dr=0 ----
                am = mid.tile([P, NT7, L], bf16, tag="am", name="am")

                def a_phase(dr, qs, ks, am):
                    for j in range(2):
                        jo, jw = ACH[j]
                        for si in range(NT7):
                            d = si - 4 * j
                            if dr == 0:
                                if si * P > jo + jw - 1:
                                    continue
                                mi_ = None if d < 0 else d
                            else:
                                if si * P + SW[si] - 1 < jo:
                                    continue
                                mi_ = None if d >= 4 else 4 + d
                            sw = SW[si]
                            pt = psum(sw, jw)
                            for ct in range(2):
                                nc.tensor.matmul(pt, ks[:, ct, si * P: si * P + sw],
                                                 qs[:, ct, jo: jo + jw],
                                                 start=(ct == 0), stop=(ct == 1))
                            if mi_ is None:
                                nc.vector.tensor_copy(am[:sw, si, jo: jo + jw], pt)
                            else:
                                nc.vector.tensor_mul(am[:sw, si, jo: jo + jw], pt,
                                                     masks[:sw, mi_, :jw])

                a_phase(0, qsf, ksf, am)

                # ---- v projection (token-major) ----
                vh = mid.tile([P, NT7, HDV], bf16, tag="vh", name="vh")
                for tt in range(NT7):
                    tw = TW[tt]
                    pt = psum(tw, HDV)
                    for kt in range(8):
                        nc.tensor.matmul(pt, xc[:, kt, bi * L + tt * P: bi * L + tt * P + tw],
                                         wcat[:, kt, 512:1024], start=(kt == 0), stop=(kt == 7))
                    nc.vector.tensor_copy(vh[:tw, tt, :], pt)

                # ---- o phase dr=0 ----
                ofn = mid.tile([P, NT7, HDV], bf16, tag="ofn", name="ofn")
                ssq0 = ogp.tile([P, NT7], f32, tag="ssq0", name="ssq0")
                nc.vector.memset(ssq0[:], 0.0)
                scrap = ogp.tile([P, HDV], bf16, tag="lg", name="scrap")
                for tt in range(NT7):
                    tw = TW[tt]
                    pt = psum(tw, HDV)
                    sis = list(range(0, tt + 1))
                    for ii, si in enumerate(sis):
                        nc.tensor.matmul(pt, am[:SW[si], si, tt * P: tt * P + tw],
                                         vh[:SW[si], si, :],
                                         start=(ii == 0), stop=(ii == len(sis) - 1))
                    nc.scalar.activation(scrap[:tw], pt, AF.Square,
                                         accum_out=ssq0[:tw, tt: tt + 1])
                    rsl0 = ogp.tile([P, 1], f32, tag="rsl0", name="rsl0")
                    nc.scalar.activation(rsl0[:tw], ssq0[:tw, tt: tt + 1], AF.Ln,
                                         scale=1.0 / HDV, bias=epst[:tw])
                    nc.scalar.activation(rsl0[:tw], rsl0[:tw], AF.Exp, scale=-0.5)
                    nc.vector.tensor_scalar_mul(ofn[:tw, tt, :], pt, rsl0[:tw])

                # ---- A + o phase dr=1 ----
                am1 = mid.tile([P, NT7, L], bf16, tag="am", name="am1")
                a_phase(1, qsb, ksb, am1)

                o1 = mid.tile([P, NT7, HDV], bf16, tag="o1", name="o1")
                ssq1 = ogp.tile([P, NT7], f32, tag="ssq1", name="ssq1")
                nc.vector.memset(ssq1[:], 0.0)
                for tt in range(NT7):
                    tw = TW[tt]
                    pt = psum(tw, HDV)
                    sis = list(range(tt, NT7))
                    for ii, si in enumerate(sis):
                        nc.tensor.matmul(pt, am1[:SW[si], si, tt * P: tt * P + tw],
                                         vh[:SW[si], si, :],
                                         start=(ii == 0), stop=(ii == len(sis) - 1))
                    nc.scalar.activation(scrap[:tw], pt, AF.Square,
                                         accum_out=ssq1[:tw, tt: tt + 1])
                    nc.vector.tensor_copy(o1[:tw, tt, :], pt)
                rsl1 = ogp.tile([P, NT7], f32, tag="rsl1", name="rsl1")
                nc.scalar.activation(rsl1, ssq1, AF.Ln, scale=1.0 / HDV, bias=epst)
                nc.scalar.activation(rsl1, rsl1, AF.Exp, scale=-0.5)

                # ---- finalize: og = (o1*rsl1 + ofn) * silu(gate), transpose ----
                ogT = ogTp.tile([P, 4, L], bf16, tag="ogT", name="ogT")
                ogTs[(bi, h)] = ogT
                og = mid.tile([P, NT7, HDV], bf16, tag="og", name="og")
                for tt in range(NT7):
                    tw = TW[tt]
                    gpt = psum(tw, HDV)
                    for kt in range(8):
                        nc.tensor.matmul(gpt, xc[:, kt, bi * L + tt * P: bi * L + tt * P + tw],
                                         gw[:, kt, :], start=(kt == 0), stop=(kt == 7))
                    eg = ogp.tile([P, HDV], bf16, tag="eg", name="eg")
                    nc.scalar.activation(eg[:tw], gpt, AF.Exp, scale=-1.0)
                    lg = ogp.tile([P, HDV], bf16, tag="lg", name="lg")
                    nc.scalar.activation(lg[:tw], eg[:tw], AF.Ln, bias=onet[:tw])
                    sg = ogp.tile([P, HDV], bf16, tag="ob", name="sg")
                    nc.scalar.activation(sg[:tw], lg[:tw], AF.Exp, scale=-1.0)
                    gv = ogp.tile([P, HDV], bf16, tag="eg", name="gv")
                    nc.vector.tensor_mul(gv[:tw], gpt, sg[:tw])
                    ob = ogp.tile([P, HDV], bf16, tag="ob", name="ob")
                    nc.vector.scalar_tensor_tensor(ob[:tw], o1[:tw, tt, :],
                                                   rsl1[:tw, tt: tt + 1],
                                                   ofn[:tw, tt, :], ALU.mult, ALU.add)
                    nc.vector.tensor_mul(og[:tw, tt, :], ob[:tw], gv[:tw])
                    tpt = tps.tile([P, 4, P], bf16, tag="tps", name="tpt")
                    for j in range(4):
                        nc.tensor.transpose(tpt[:, j, :tw], og[:tw, tt, j * P:(j + 1) * P],
                                            ident[:tw, :tw])
                    nc.vector.tensor_copy(ogT[:, :, tt * P: tt * P + tw], tpt[:, :, :tw])

                # ---- Stage F: out projection for this bi ----
                if h == 3:
                    for nch in range(2):
                        if nch == 0:
                            halves = owh0
                        else:
                            halves = []
                            for hf in range(2):
                                owh = bsl.tile([P, 8, 512], bf16, tag="bslab", name="owh")
                                nc.gpsimd.dma_start(
                                    out=owh,
                                    in_=ow_d[hf * 8:(hf + 1) * 8, :, 512:1024]
                                    .rearrange("j p c -> p j c"))
                                halves.append(owh)
                        for tt in range(NT7):
                            tw = TW[tt]
                            pt = psum(tw, 512)
                            for jt in range(16):
                                h_, j_ = divmod(jt, 4)
                                nc.tensor.matmul(pt,
                                                 ogTs[(bi, h_)][:, j_, tt * P: tt * P + tw],
                                                 halves[jt // 8][:, jt % 8, :],
                                                 start=(jt == 0), stop=(jt == 15))
                            outs = outp.tile([P, 512], f32, tag="outs", name="outs")
                            nc.vector.tensor_copy(outs[:tw, :], pt)
                            nc.sync.dma_start(
                                out=out_d[bi * L + tt * P: bi * L + tt * P + tw,
                                          nch * 512:(nch + 1) * 512],
                                in_=outs[:tw, :])

    _legalize_sync_waits(nc)
    return nc


# revision 3
# speedup vs baseline: 1.0204x; 1.0066x over previous
"""Trainium2 Bass kernel for nn_GatedLinearAttention (bidirectional GLA vision block), v2.

Same math as baseline (quadratic masked attention with global decay), restructured
for PE continuity:
  - one activation-table epoch switch total: Silu (conv) -> natural_log_exp for the
    rest: log_sigmoid(u) = -ln(1 + exp(-u)) (Exp then Ln with bias=1), gates via
    silu(g) = g / (1 + exp(-g)) (scalar Exp + DVE reciprocal), rsqrt via Ln+Exp.
  - decay chains computed one combo ahead (scalar/gpsimd/DVE work hidden under PE).
  - cumsum scans + reverse fixes moved to the idle GpSimd engine.
  - og transposed via PE identity-matmul transposes (not serialized DMA transposes).
  - out projection reads transposed og tiles; weights streamed per half-slab.
Data-parallel over batch: 16 batch items -> 8 cores x 2. No collectives.
"""

import os
import sys
from contextlib import ExitStack

for _p in ("/opt/trn_rl_repo", "/root/.axon_site/_ro/trn_rl_repo"):
    if os.path.isdir(_p) and _p not in sys.path:
        sys.path.insert(0, _p)

import numpy as np
import ml_dtypes

import concourse.bass as bass
import concourse.tile as tile
import concourse.mybir as mybir
from concourse.bass_utils import run_bass_kernel_spmd

f32 = mybir.dt.float32
bf16 = mybir.dt.bfloat16
AF = mybir.ActivationFunctionType
ALU = mybir.AluOpType

P = 128
NCORES = 8
B = 2               # batch items per core
L = 784             # tokens per batch item
T = B * L
D = 1024
NH = 4
HDK = 256
HDV = 512
GLN = 16.0
EPS = 1e-5
NT7 = 7
TW = [128, 128, 128, 128, 128, 128, 16]
SW = TW
TC2 = [(0, 392), (392, 392)]
ACH = [(0, 512), (512, 272)]


def _legalize_sync_waits(nc, max_waits=1):
    """Split >1 semaphore waits onto chained NOPs (walrus limitation)."""
    counter = 0
    for fn in nc.m.functions:
        for blk in fn.blocks:
            insts = list(blk.instructions)
            changed = False
            out = []
            for inst in insts:
                si = inst.sync_info
                if si is not None and len(si.on_wait) > max_waits:
                    waits = list(si.on_wait)
                    keep = waits[len(waits) - max_waits:]
                    move = waits[: len(waits) - max_waits]
                    for i in range(0, len(move), max_waits):
                        chunk = move[i: i + max_waits]
                        nop = mybir.InstNoOp(
                            name=f"legalize-wait-nop-{counter}", ins=[], outs=[]
                        )
                        counter += 1
                        nop.engine = inst.engine
                        nop.sync_info = mybir.SyncInfo(on_wait=chunk, on_update=[])
                        out.append(nop)
                    inst.sync_info = mybir.SyncInfo(
                        on_wait=keep, on_update=list(si.on_update)
                    )
                    changed = True
                out.append(inst)
            if changed:
                blk.instructions = out


def _build_program():
    nc = bass.Bass()

    xpad_d = nc.dram_tensor("xpad", [8, P, B * 30 * 30], bf16, kind="ExternalInput")
    cw_d = nc.dram_tensor("cw", [8, P, 9], f32, kind="ExternalInput")
    wcat_d = nc.dram_tensor("wcat", [NH, 8, P, 1024], bf16, kind="ExternalInput")
    gwc_d = nc.dram_tensor("gwc", [NH, 8, P, 512], bf16, kind="ExternalInput")
    gk1w_d = nc.dram_tensor("gk1w", [8, P, 16], bf16, kind="ExternalInput")
    gk2w_d = nc.dram_tensor("gk2w", [16, 2048], bf16, kind="ExternalInput")
    b2n_d = nc.dram_tensor("b2n", [16, P, 1], f32, kind="ExternalInput")
    ow_d = nc.dram_tensor("ow", [16, P, 1024], bf16, kind="ExternalInput")
    f8 = mybir.dt.float8e4
    masks_d = nc.dram_tensor("masks", [8, P, 512], f8, kind="ExternalInput")
    ident_d = nc.dram_tensor("ident", [P, P], bf16, kind="ExternalInput")
    out_d = nc.dram_tensor("out", [T, 1024], f32, kind="ExternalOutput")

    with tile.TileContext(nc) as tc:
        with ExitStack() as ctx:
            cst = ctx.enter_context(tc.tile_pool(name="cst", bufs=1))
            big = ctx.enter_context(tc.tile_pool(name="big", bufs=1))
            wc = ctx.enter_context(tc.tile_pool(name="wc", bufs=2))
            gwp = ctx.enter_context(tc.tile_pool(name="gwp", bufs=1))
            bsl = ctx.enter_context(tc.tile_pool(name="bsl", bufs=2))
            dec = ctx.enter_context(tc.tile_pool(name="dec", bufs=2))
            cpp = ctx.enter_context(tc.tile_pool(name="cpp", bufs=1))
            qk = ctx.enter_context(tc.tile_pool(name="qk", bufs=1))
            mid = ctx.enter_context(tc.tile_pool(name="mid", bufs=1))
            ogp = ctx.enter_context(tc.tile_pool(name="ogp", bufs=1))
            ogTp = ctx.enter_context(tc.tile_pool(name="ogTp", bufs=4))
            sout = ctx.enter_context(tc.tile_pool(name="sout", bufs=2))
            outp = ctx.enter_context(tc.tile_pool(name="outp", bufs=2))
            xpp = ctx.enter_context(tc.tile_pool(name="xpp", bufs=2))
            cdp = ctx.enter_context(tc.tile_pool(name="cdp", bufs=2))
            ps = ctx.enter_context(tc.tile_pool(name="ps", bufs=6, space="PSUM"))
            tps = ctx.enter_context(tc.tile_pool(name="tps", bufs=2, space="PSUM"))

            def psum(rows, cols):
                pstile = ps.tile([P, 512], f32, tag="ps", name="pstile")
                return pstile[:rows, :cols]

            # ---- constants ----
            wvec = cst.tile([P, 8, 9], f32)
            nc.gpsimd.dma_start(out=wvec, in_=cw_d.rearrange("f p t -> p f t"))
            ident = cst.tile([P, P], bf16)
            nc.gpsimd.dma_start(out=ident, in_=ident_d[:])
            masks = cst.tile([P, 8, 512], f8)
            nc.gpsimd.dma_start(out=masks, in_=masks_d.rearrange("m p t -> p m t"))
            zeros = cst.tile([P, 392], bf16)
            nc.vector.memset(zeros[:], 0.0)
            epsone = cst.tile([P, 2], f32)
            nc.vector.memset(epsone[:, 0:1], EPS)
            nc.vector.memset(epsone[:, 1:2], 1.0)
            epst = epsone[:, 0:1]
            onet = epsone[:, 1:2]
            w1 = cst.tile([P, 8, 16], bf16)
            nc.gpsimd.dma_start(out=w1, in_=gk1w_d.rearrange("k p c -> p k c"))

            # ---- persistent activations ----
            xc = big.tile([P, 8, T], bf16)
            gk1o = big.tile([16, T], bf16)

            # ==== Stage A+B per batch item: conv 3x3 + silu (PE diag), gk1 ====
            def stage_ab(bi):
                for ft in range(8):
                    xp = xpp.tile([P, 30, 30], bf16, tag="xp", name="xp")
                    nc.gpsimd.dma_start(
                        out=xp, in_=xpad_d[ft].rearrange("p (b h w) -> p b h w", b=B, h=30)[:, bi])
                    cd = cdp.tile([P, 9, P], bf16, tag="cd", name="cd")
                    for tap in range(9):
                        nc.vector.tensor_scalar_mul(cd[:, tap, :], ident[:],
                                                    wvec[:, ft, tap: tap + 1])
                    pts = [psum(P, 392) for _ in range(2)]
                    for tap in range(9):
                        a, bb = tap // 3, tap % 3
                        for half in range(2):
                            rhs = xp[:, a + half * 14: a + half * 14 + 14, bb: bb + 28]
                            nc.tensor.matmul(pts[half], cd[:, tap, :], rhs,
                                             start=(tap == 0), stop=(tap == 8))
                    for half in range(2):
                        dst = xc[:, ft, bi * L + half * 392: bi * L + (half + 1) * 392]
                        nc.scalar.activation(dst, pts[half], AF.Silu)
                for tc2 in range(2):
                    pt = psum(16, 392)
                    for kt in range(8):
                        nc.tensor.matmul(pt, w1[:, kt, :],
                                         xc[:, kt, bi * L + tc2 * 392: bi * L + (tc2 + 1) * 392],
                                         start=(kt == 0), stop=(kt == 7))
                    nc.vector.tensor_copy(gk1o[:, bi * L + tc2 * 392: bi * L + (tc2 + 1) * 392], pt)

            # ============== decay chains (one combo of lookahead) ==============
            def emit_decay(c):
                bi, h = divmod(c, 4)
                w2 = cpp.tile([16, 4, P], bf16, tag="w2", name="w2")
                nc.gpsimd.dma_start(out=w2[:, 0:2, :],
                                  in_=gk2w_d[:, h * HDK:(h + 1) * HDK].rearrange("k (c p) -> k c p", c=2))
                nc.gpsimd.dma_start(out=w2[:, 2:4, :],
                                  in_=gk2w_d[:, 1024 + h * HDK: 1024 + (h + 1) * HDK].rearrange("k (c p) -> k c p", c=2))
                b2t = dec.tile([P, 4], f32, tag="b2t", name="b2t")
                for mi, mt in enumerate([2 * h, 2 * h + 1, 8 + 2 * h, 8 + 2 * h + 1]):
                    nc.gpsimd.dma_start(out=b2t[:, mi: mi + 1], in_=b2n_d[mt])
                eqf = dec.tile([P, 2, L], bf16, tag="eqf", name="eqf", bufs=1)
                eqb = dec.tile([P, 2, L], bf16, tag="eqb", name="eqb", bufs=1)
                ekf = dec.tile([P, 2, L], bf16, tag="ekf", name="ekf", bufs=1)
                ekb = dec.tile([P, 2, L], bf16, tag="ekb", name="ekb", bufs=1)
                for dr in range(2):
                    et = dec.tile([P, 2, L], bf16, tag="edec", name="et")
                    lp = dec.tile([P, 2, L], bf16, tag="edec", name="lp")
                    cp = cpp.tile([P, 2, L], f32, tag="cp", name="cp")
                    for ct in range(2):
                        mi = dr * 2 + ct
                        for o0, w0 in TC2:
                            upt = psum(P, 392)
                            nc.tensor.matmul(upt, w2[:, mi, :],
                                             gk1o[:, bi * L + o0: bi * L + o0 + w0],
                                             start=True, stop=True)
                            # e = exp(-(u0 + b2)) = exp(-u0 + b2neg)
                            nc.scalar.activation(et[:, ct, o0:o0 + w0], upt, AF.Exp,
                                                 scale=-1.0, bias=b2t[:, mi: mi + 1])
                        # lp = ln(1 + e) = softplus(-u) = -log_sigmoid(u)
                        nc.scalar.activation(lp[:, ct, :], et[:, ct, :], AF.Ln, bias=onet)
                        nc.vector.tensor_tensor_scan(cp[:, ct, 0:392], lp[:, ct, 0:392],
                                                     zeros[:], 0.0, ALU.add, ALU.add)
                        nc.vector.tensor_tensor_scan(cp[:, ct, 392:L], lp[:, ct, 392:L],
                                                     zeros[:], cp[:, ct, 391:392],
                                                     ALU.add, ALU.add)
                        if dr == 1:
                            # cpr = lp - cp + cp_total (reverse-inclusive cumsum)
                            tot = dec.tile([P, 1], f32, tag="tot", name="tot")
                            nc.vector.tensor_copy(tot[:, 0:1], cp[:, ct, L - 1:L])
                            nc.vector.tensor_sub(cp[:, ct, :], lp[:, ct, :], cp[:, ct, :])
                            nc.vector.tensor_scalar_add(cp[:, ct, :], cp[:, ct, :], tot[:, 0:1])
                    dq = eqf if dr == 0 else eqb
                    dk = ekf if dr == 0 else ekb
                    nc.scalar.activation(dq.rearrange("p a b -> p (a b)"),
                                         cp.rearrange("p a b -> p (a b)"),
                                         AF.Exp, scale=-1.0 / GLN)
                    nc.scalar.activation(dk.rearrange("p a b -> p (a b)"),
                                         cp.rearrange("p a b -> p (a b)"),
                                         AF.Exp, scale=1.0 / GLN)
                return eqf, eqb, ekf, ekb

            # prefetch weights for combo 0
            def load_wcat(h):
                w = wc.tile([P, 8, 1024], bf16, tag="wcat", name="wcat")
                nc.gpsimd.dma_start(out=w, in_=wcat_d[h].rearrange("k p c -> p k c"))
                return w

            def load_gw(h):
                g = gwp.tile([P, 8, 512], bf16, tag="gw", name="gw")
                nc.gpsimd.dma_start(out=g, in_=gwc_d[h].rearrange("k p c -> p k c"))
                return g

            stage_ab(0)
            wcats = {0: load_wcat(0)}
            gws = {0: load_gw(0)}
            stage_ab(1)
            decays = {0: emit_decay(0)}
            ogTs = {}

            for c in range(8):
                bi, h = divmod(c, 4)
                eqf, eqb, ekf, ekb = decays.pop(c)
                wcat = wcats.pop(c)
                gw = gws.pop(c)
                if c + 1 < 8:
                    wcats[c + 1] = load_wcat((c + 1) % 4)
                    gws[c + 1] = load_gw((c + 1) % 4)
                owh0 = []
                if h == 3:
                    for hf in range(2):
                        owh = bsl.tile([P, 8, 512], bf16, tag="bslab", name="owh")
                        nc.gpsimd.dma_start(
                            out=owh,
                            in_=ow_d[hf * 8:(hf + 1) * 8, :, 0:512]
                            .rearrange("j p c -> p j c"))
                        owh0.append(owh)

                # ---- q/k projections + decay muls (feature-major [feat, tok]) ----
                qsf = qk.tile([P, 2, L], bf16, tag="qsf", name="qsf")
                qsb = qk.tile([P, 2, L], bf16, tag="qsb", name="qsb")
                ksf = qk.tile([P, 2, L], bf16, tag="ksf", name="ksf")
                ksb = qk.tile([P, 2, L], bf16, tag="ksb", name="ksb")
                for ct in range(2):
                    for o0, w0 in TC2:
                        sl = slice(o0, o0 + w0)
                        qpt = psum(P, 392)
                        for kt in range(8):
                            nc.tensor.matmul(qpt, wcat[:, kt, ct * P:(ct + 1) * P],
                                             xc[:, kt, bi * L + o0: bi * L + o0 + w0],
                                             start=(kt == 0), stop=(kt == 7))
                        nc.vector.tensor_mul(qsf[:, ct, sl], qpt, eqf[:, ct, sl])
                        nc.vector.tensor_mul(qsb[:, ct, sl], qpt, eqb[:, ct, sl])
                        kpt = psum(P, 392)
                        for kt in range(8):
                            nc.tensor.matmul(kpt, wcat[:, kt, 256 + ct * P: 256 + (ct + 1) * P],
                                             xc[:, kt, bi * L + o0: bi * L + o0 + w0],
                                             start=(kt == 0), stop=(kt == 7))
                        nc.vector.tensor_mul(ksf[:, ct, sl], kpt, ekf[:, ct, sl])
                        nc.vector.tensor_mul(ksb[:, ct, sl], kpt, ekb[:, ct, sl])

                if c + 1 < 8:
                    decays[c + 1] = emit_decay(c + 1)

                # ---- A phase dr=0 ----
                am = mid.tile([P, NT7, L], bf16, tag="am", name="am")

                def a_phase(dr, qs, ks, am):
                    for j in range(2):
                        jo, jw = ACH[j]
                        for si in range(NT7):
                            d = si - 4 * j
                            if dr == 0:
                                if si * P > jo + jw - 1:
                                    continue
                                mi_ = None if d < 0 else d
                            else:
                                if si * P + SW[si] - 1 < jo:
                                    continue
                                mi_ = None if d >= 4 else 4 + d
                            sw = SW[si]
                            pt = psum(sw, jw)
                            for ct in range(2):
                                nc.tensor.matmul(pt, ks[:, ct, si * P: si * P + sw],
                                                 qs[:, ct, jo: jo + jw],
                                                 start=(ct == 0), stop=(ct == 1))
                            if mi_ is None:
                                nc.vector.tensor_copy(am[:sw, si, jo: jo + jw], pt)
                            else:
                                nc.vector.tensor_mul(am[:sw, si, jo: jo + jw], pt,
                                                     masks[:sw, mi_, :jw])

                a_phase(0, qsf, ksf, am)

                # ---- v projection (token-major) ----
                vh = mid.tile([P, NT7, HDV], bf16, tag="vh", name="vh")
                for tt in range(NT7):
                    tw = TW[tt]
                    pt = psum(tw, HDV)
                    for kt in range(8):
                        nc.tensor.matmul(pt, xc[:, kt, bi * L + tt * P: bi * L + tt * P + tw],
                                         wcat[:, kt, 512:1024], start=(kt == 0), stop=(kt == 7))
                    nc.vector.tensor_copy(vh[:tw, tt, :], pt)

                # ---- o phase dr=0 ----
                ofn = mid.tile([P, NT7, HDV], bf16, tag="ofn", name="ofn")
                ssq0 = ogp.tile([P, NT7], f32, tag="ssq0", name="ssq0")
                nc.vector.memset(ssq0[:], 0.0)
                scrap = ogp.tile([P, HDV], bf16, tag="lg", name="scrap")
                for tt in range(NT7):
                    tw = TW[tt]
                    pt = psum(tw, HDV)
                    sis = list(range(0, tt + 1))
                    for ii, si in enumerate(sis):
                        nc.tensor.matmul(pt, am[:SW[si], si, tt * P: tt * P + tw],
                                         vh[:SW[si], si, :],
                                         start=(ii == 0), stop=(ii == len(sis) - 1))
                    nc.scalar.activation(scrap[:tw], pt, AF.Square,
                                         accum_out=ssq0[:tw, tt: tt + 1])
                    rsl0 = ogp.tile([P, 1], f32, tag="rsl0", name="rsl0")
                    nc.scalar.activation(rsl0[:tw], ssq0[:tw, tt: tt + 1], AF.Ln,
                                         scale=1.0 / HDV, bias=epst[:tw])
                    nc.scalar.activation(rsl0[:tw], rsl0[:tw], AF.Exp, scale=-0.5)
                    nc.vector.tensor_scalar_mul(ofn[:tw, tt, :], pt, rsl0[:tw])

                # ---- A + o phase dr=1 ----
                am1 = mid.tile([P, NT7, L], bf16, tag="am", name="am1")
                a_phase(1, qsb, ksb, am1)

                o1 = mid.tile([P, NT7, HDV], bf16, tag="o1", name="o1")
                ssq1 = ogp.tile([P, NT7], f32, tag="ssq1", name="ssq1")
                nc.vector.memset(ssq1[:], 0.0)
                for tt in range(NT7):
                    tw = TW[tt]
                    pt = psum(tw, HDV)
                    sis = list(range(tt, NT7))
                    for ii, si in enumerate(sis):
                        nc.tensor.matmul(pt, am1[:SW[si], si, tt * P: tt * P + tw],
                                         vh[:SW[si], si, :],
                                         start=(ii == 0), stop=(ii == len(sis) - 1))
                    nc.scalar.activation(scrap[:tw], pt, AF.Square,
                                         accum_out=ssq1[:tw, tt: tt + 1])
                    nc.vector.tensor_copy(o1[:tw, tt, :], pt)
                rsl1 = ogp.tile([P, NT7], f32, tag="rsl1", name="rsl1")
                nc.scalar.activation(rsl1, ssq1, AF.Ln, scale=1.0 / HDV, bias=epst)
                nc.scalar.activation(rsl1, rsl1, AF.Exp, scale=-0.5)

                # ---- finalize: og = (o1*rsl1 + ofn) * silu(gate), transpose ----
                ogT = ogTp.tile([P, 4, L], bf16, tag="ogT", name="ogT")
                ogTs[(bi, h)] = ogT
                og = mid.tile([P, NT7, HDV], bf16, tag="og", name="og")
                for tt in range(NT7):
                    tw = TW[tt]
                    gpt = psum(tw, HDV)
                    for kt in range(8):
                        nc.tensor.matmul(gpt, xc[:, kt, bi * L + tt * P: bi * L + tt * P + tw],
                                         gw[:, kt, :], start=(kt == 0), stop=(kt == 7))
                    eg = ogp.tile([P, HDV], bf16, tag="eg", name="eg")
                    nc.scalar.activation(eg[:tw], gpt, AF.Exp, scale=-1.0)
                    lg = ogp.tile([P, HDV], bf16, tag="lg", name="lg")
                    nc.scalar.activation(lg[:tw], eg[:tw], AF.Ln, bias=onet[:tw])
                    sg = ogp.tile([P, HDV], bf16, tag="sg", name="sg")
                    nc.scalar.activation(sg[:tw], lg[:tw], AF.Exp, scale=-1.0)
                    gv = ogp.tile([P, HDV], bf16, tag="eg", name="gv")
                    nc.vector.tensor_mul(gv[:tw], gpt, sg[:tw])
                    ob = ogp.tile([P, HDV], bf16, tag="ob", name="ob")
                    nc.vector.scalar_tensor_tensor(ob[:tw], o1[:tw, tt, :],
                                                   rsl1[:tw, tt: tt + 1],
                                                   ofn[:tw, tt, :], ALU.mult, ALU.add)
                    nc.vector.tensor_mul(og[:tw, tt, :], ob[:tw], gv[:tw])
                    tpt = tps.tile([P, 4, P], bf16, tag="tps", name="tpt")
                    for j in range(4):
                        nc.tensor.transpose(tpt[:, j, :tw], og[:tw, tt, j * P:(j + 1) * P],
                                            ident[:tw, :tw])
                    nc.vector.tensor_copy(ogT[:, :, tt * P: tt * P + tw], tpt[:, :, :tw])

                # ---- Stage F: out projection for this bi ----
                if h == 3:
                    for nch in range(2):
                        if nch == 0:
                            halves = owh0
                        else:
                            halves = []
                            for hf in range(2):
                                owh = bsl.tile([P, 8, 512], bf16, tag="bslab", name="owh")
                                nc.gpsimd.dma_start(
                                    out=owh,
                                    in_=ow_d[hf * 8:(hf + 1) * 8, :, 512:1024]
                                    .rearrange("j p c -> p j c"))
                                halves.append(owh)
                        for tt in range(NT7):
                            tw = TW[tt]
                            pt = psum(tw, 512)
                            for jt in range(16):
                                h_, j_ = divmod(jt, 4)
                                nc.tensor.matmul(pt,
                                                 ogTs[(bi, h_)][:, j_, tt * P: tt * P + tw],
                                                 halves[jt // 8][:, jt % 8, :],
                                                 start=(jt == 0), stop=(jt == 15))
                            outs = outp.tile([P, 512], f32, tag="outs", name="outs")
                            nc.vector.tensor_copy(outs[:tw, :], pt)
                            nc.sync.dma_start(
                                out=out_d[bi * L + tt * P: bi * L + tt * P + tw,
                                          nch * 512:(nch + 1) * 512],
                                in_=outs[:tw, :])

    _legalize_sync_waits(nc)
    return nc


# revision 4
# speedup vs baseline: 1.0272x; 1.0066x over previous
"""Trainium2 Bass kernel for nn_GatedLinearAttention (bidirectional GLA vision block), v2.

Same math as baseline (quadratic masked attention with global decay), restructured
for PE continuity:
  - one activation-table epoch switch total: Silu (conv) -> natural_log_exp for the
    rest: log_sigmoid(u) = -ln(1 + exp(-u)) (Exp then Ln with bias=1), gates via
    silu(g) = g / (1 + exp(-g)) (scalar Exp + DVE reciprocal), rsqrt via Ln+Exp.
  - decay chains computed one combo ahead (scalar/gpsimd/DVE work hidden under PE).
  - cumsum scans + reverse fixes moved to the idle GpSimd engine.
  - og transposed via PE identity-matmul transposes (not serialized DMA transposes).
  - out projection reads transposed og tiles; weights streamed per half-slab.
Data-parallel over batch: 16 batch items -> 8 cores x 2. No collectives.
"""

import os
import sys
from contextlib import ExitStack

for _p in ("/opt/trn_rl_repo", "/root/.axon_site/_ro/trn_rl_repo"):
    if os.path.isdir(_p) and _p not in sys.path:
        sys.path.insert(0, _p)

import numpy as np
import ml_dtypes

import concourse.bass as bass
import concourse.tile as tile
import concourse.mybir as mybir
from concourse.bass_utils import run_bass_kernel_spmd

f32 = mybir.dt.float32
bf16 = mybir.dt.bfloat16
AF = mybir.ActivationFunctionType
ALU = mybir.AluOpType

P = 128
NCORES = 8
B = 2               # batch items per core
L = 784             # tokens per batch item
T = B * L
D = 1024
NH = 4
HDK = 256
HDV = 512
GLN = 16.0
EPS = 1e-5
NT7 = 7
TW = [128, 128, 128, 128, 128, 128, 16]
SW = TW
TC2 = [(0, 392), (392, 392)]
ACH = [(0, 512), (512, 272)]


def _legalize_sync_waits(nc, max_waits=1):
    """Split >1 semaphore waits onto chained NOPs (walrus limitation)."""
    counter = 0
    for fn in nc.m.functions:
        for blk in fn.blocks:
            insts = list(blk.instructions)
            changed = False
            out = []
            for inst in insts:
                si = inst.sync_info
                if si is not None and len(si.on_wait) > max_waits:
                    waits = list(si.on_wait)
                    keep = waits[len(waits) - max_waits:]
                    move = waits[: len(waits) - max_waits]
                    for i in range(0, len(move), max_waits):
                        chunk = move[i: i + max_waits]
                        nop = mybir.InstNoOp(
                            name=f"legalize-wait-nop-{counter}", ins=[], outs=[]
                        )
                        counter += 1
                        nop.engine = inst.engine
                        nop.sync_info = mybir.SyncInfo(on_wait=chunk, on_update=[])
                        out.append(nop)
                    inst.sync_info = mybir.SyncInfo(
                        on_wait=keep, on_update=list(si.on_update)
                    )
                    changed = True
                out.append(inst)
            if changed:
                blk.instructions = out


def _build_program():
    nc = bass.Bass()

    xpad_d = nc.dram_tensor("xpad", [8, P, B * 30 * 30], bf16, kind="ExternalInput")
    cw_d = nc.dram_tensor("cw", [8, P, 9], f32, kind="ExternalInput")
    wcat_d = nc.dram_tensor("wcat", [NH, 8, P, 1024], bf16, kind="ExternalInput")
    gwc_d = nc.dram_tensor("gwc", [NH, 8, P, 512], bf16, kind="ExternalInput")
    gk1w_d = nc.dram_tensor("gk1w", [8, P, 16], bf16, kind="ExternalInput")
    gk2w_d = nc.dram_tensor("gk2w", [16, 2048], bf16, kind="ExternalInput")
    b2n_d = nc.dram_tensor("b2n", [16, P, 1], f32, kind="ExternalInput")
    ow_d = nc.dram_tensor("ow", [16, P, 1024], bf16, kind="ExternalInput")
    f8 = mybir.dt.float8e4
    masks_d = nc.dram_tensor("masks", [8, P, 512], f8, kind="ExternalInput")
    ident_d = nc.dram_tensor("ident", [P, P], bf16, kind="ExternalInput")
    out_d = nc.dram_tensor("out", [T, 1024], f32, kind="ExternalOutput")

    with tile.TileContext(nc) as tc:
        with ExitStack() as ctx:
            cst = ctx.enter_context(tc.tile_pool(name="cst", bufs=1))
            big = ctx.enter_context(tc.tile_pool(name="big", bufs=1))
            wc = ctx.enter_context(tc.tile_pool(name="wc", bufs=2))
            gwp = ctx.enter_context(tc.tile_pool(name="gwp", bufs=1))
            bsl = ctx.enter_context(tc.tile_pool(name="bsl", bufs=2))
            dec = ctx.enter_context(tc.tile_pool(name="dec", bufs=2))
            cpp = ctx.enter_context(tc.tile_pool(name="cpp", bufs=1))
            qk = ctx.enter_context(tc.tile_pool(name="qk", bufs=1))
            mid = ctx.enter_context(tc.tile_pool(name="mid", bufs=1))
            ogp = ctx.enter_context(tc.tile_pool(name="ogp", bufs=1))
            ogTp = ctx.enter_context(tc.tile_pool(name="ogTp", bufs=4))
            sout = ctx.enter_context(tc.tile_pool(name="sout", bufs=2))
            outp = ctx.enter_context(tc.tile_pool(name="outp", bufs=2))
            xpp = ctx.enter_context(tc.tile_pool(name="xpp", bufs=2))
            cdp = ctx.enter_context(tc.tile_pool(name="cdp", bufs=2))
            ps = ctx.enter_context(tc.tile_pool(name="ps", bufs=6, space="PSUM"))
            tps = ctx.enter_context(tc.tile_pool(name="tps", bufs=2, space="PSUM"))

            def psum(rows, cols):
                pstile = ps.tile([P, 512], f32, tag="ps", name="pstile")
                return pstile[:rows, :cols]

            # ---- constants ----
            wvec = cst.tile([P, 8, 9], f32)
            nc.gpsimd.dma_start(out=wvec, in_=cw_d.rearrange("f p t -> p f t"))
            ident = cst.tile([P, P], bf16)
            nc.gpsimd.dma_start(out=ident, in_=ident_d[:])
            zeros = cst.tile([P, 392], bf16)
            nc.vector.memset(zeros[:], 0.0)
            epsone = cst.tile([P, 2], f32)
            nc.vector.memset(epsone[:, 0:1], EPS)
            nc.vector.memset(epsone[:, 1:2], 1.0)
            epst = epsone[:, 0:1]
            onet = epsone[:, 1:2]
            w1 = cst.tile([P, 8, 16], bf16)
            nc.gpsimd.dma_start(out=w1, in_=gk1w_d.rearrange("k p c -> p k c"))

            # ---- persistent activations ----
            xc = big.tile([P, 8, T], bf16)
            gk1o = big.tile([16, T], bf16)

            # ==== Stage A+B per batch item: conv 3x3 + silu (PE diag), gk1 ====
            def stage_ab(bi):
                for ft in range(8):
                    xp = xpp.tile([P, 30, 30], bf16, tag="xp", name="xp")
                    nc.gpsimd.dma_start(
                        out=xp, in_=xpad_d[ft].rearrange("p (b h w) -> p b h w", b=B, h=30)[:, bi])
                    cd = cdp.tile([P, 9, P], bf16, tag="cd", name="cd")
                    for tap in range(9):
                        nc.vector.tensor_scalar_mul(cd[:, tap, :], ident[:],
                                                    wvec[:, ft, tap: tap + 1])
                    pts = [psum(P, 392) for _ in range(2)]
                    for tap in range(9):
                        a, bb = tap // 3, tap % 3
                        for half in range(2):
                            rhs = xp[:, a + half * 14: a + half * 14 + 14, bb: bb + 28]
                            nc.tensor.matmul(pts[half], cd[:, tap, :], rhs,
                                             start=(tap == 0), stop=(tap == 8))
                    for half in range(2):
                        dst = xc[:, ft, bi * L + half * 392: bi * L + (half + 1) * 392]
                        nc.scalar.activation(dst, pts[half], AF.Silu)
                for tc2 in range(2):
                    pt = psum(16, 392)
                    for kt in range(8):
                        nc.tensor.matmul(pt, w1[:, kt, :],
                                         xc[:, kt, bi * L + tc2 * 392: bi * L + (tc2 + 1) * 392],
                                         start=(kt == 0), stop=(kt == 7))
                    nc.vector.tensor_copy(gk1o[:, bi * L + tc2 * 392: bi * L + (tc2 + 1) * 392], pt)

            # ============== decay chains (one combo of lookahead) ==============
            def emit_decay(c):
                bi, h = divmod(c, 4)
                w2 = cpp.tile([16, 4, P], bf16, tag="w2", name="w2")
                nc.gpsimd.dma_start(out=w2[:, 0:2, :],
                                  in_=gk2w_d[:, h * HDK:(h + 1) * HDK].rearrange("k (c p) -> k c p", c=2))
                nc.gpsimd.dma_start(out=w2[:, 2:4, :],
                                  in_=gk2w_d[:, 1024 + h * HDK: 1024 + (h + 1) * HDK].rearrange("k (c p) -> k c p", c=2))
                b2t = dec.tile([P, 4], f32, tag="b2t", name="b2t")
                for mi, mt in enumerate([2 * h, 2 * h + 1, 8 + 2 * h, 8 + 2 * h + 1]):
                    nc.gpsimd.dma_start(out=b2t[:, mi: mi + 1], in_=b2n_d[mt])
                eqf = dec.tile([P, 2, L], bf16, tag="eqf", name="eqf", bufs=1)
                eqb = dec.tile([P, 2, L], bf16, tag="eqb", name="eqb", bufs=1)
                ekf = dec.tile([P, 2, L], bf16, tag="ekf", name="ekf", bufs=1)
                ekb = dec.tile([P, 2, L], bf16, tag="ekb", name="ekb", bufs=1)
                for dr in range(2):
                    et = dec.tile([P, 2, L], bf16, tag="edec", name="et")
                    lp = dec.tile([P, 2, L], bf16, tag="edec", name="lp")
                    cp = cpp.tile([P, 2, L], f32, tag="cp", name="cp")
                    for ct in range(2):
                        mi = dr * 2 + ct
                        for o0, w0 in TC2:
                            upt = psum(P, 392)
                            nc.tensor.matmul(upt, w2[:, mi, :],
                                             gk1o[:, bi * L + o0: bi * L + o0 + w0],
                                             start=True, stop=True)
                            # e = exp(-(u0 + b2)) = exp(-u0 + b2neg)
                            nc.scalar.activation(et[:, ct, o0:o0 + w0], upt, AF.Exp,
                                                 scale=-1.0, bias=b2t[:, mi: mi + 1])
                        # lp = ln(1 + e) = softplus(-u) = -log_sigmoid(u)
                        nc.scalar.activation(lp[:, ct, :], et[:, ct, :], AF.Ln, bias=onet)
                        nc.vector.tensor_tensor_scan(cp[:, ct, 0:392], lp[:, ct, 0:392],
                                                     zeros[:], 0.0, ALU.add, ALU.add)
                        nc.vector.tensor_tensor_scan(cp[:, ct, 392:L], lp[:, ct, 392:L],
                                                     zeros[:], cp[:, ct, 391:392],
                                                     ALU.add, ALU.add)
                        if dr == 1:
                            # cpr = lp - cp + cp_total (reverse-inclusive cumsum)
                            tot = dec.tile([P, 1], f32, tag="tot", name="tot")
                            nc.vector.tensor_copy(tot[:, 0:1], cp[:, ct, L - 1:L])
                            nc.vector.tensor_sub(cp[:, ct, :], lp[:, ct, :], cp[:, ct, :])
                            nc.vector.tensor_scalar_add(cp[:, ct, :], cp[:, ct, :], tot[:, 0:1])
                    dq = eqf if dr == 0 else eqb
                    dk = ekf if dr == 0 else ekb
                    nc.scalar.activation(dq.rearrange("p a b -> p (a b)"),
                                         cp.rearrange("p a b -> p (a b)"),
                                         AF.Exp, scale=-1.0 / GLN)
                    nc.scalar.activation(dk.rearrange("p a b -> p (a b)"),
                                         cp.rearrange("p a b -> p (a b)"),
                                         AF.Exp, scale=1.0 / GLN)
                return eqf, eqb, ekf, ekb

            # prefetch weights for combo 0
            def load_wcat(h):
                w = wc.tile([P, 8, 1024], bf16, tag="wcat", name="wcat")
                nc.gpsimd.dma_start(out=w, in_=wcat_d[h].rearrange("k p c -> p k c"))
                return w

            def load_gw(h):
                g = gwp.tile([P, 8, 512], bf16, tag="gw", name="gw")
                nc.gpsimd.dma_start(out=g, in_=gwc_d[h].rearrange("k p c -> p k c"))
                return g

            stage_ab(0)
            wcats = {0: load_wcat(0)}
            gws = {0: load_gw(0)}
            stage_ab(1)
            masks = cst.tile([P, 8, 512], f8)
            nc.gpsimd.dma_start(out=masks, in_=masks_d.rearrange("m p t -> p m t"))

            def emit_F(fbi, owh0):
                for nch in range(2):
                    if nch == 0:
                        halves = owh0
                    else:
                        halves = []
                        for hf in range(2):
                            owh = bsl.tile([P, 8, 512], bf16, tag="bslab", name="owh")
                            nc.gpsimd.dma_start(
                                out=owh,
                                in_=ow_d[hf * 8:(hf + 1) * 8, :, 512:1024]
                                .rearrange("j p c -> p j c"))
                            halves.append(owh)
                    for tt in range(NT7):
                        tw = TW[tt]
                        pt = psum(tw, 512)
                        for jt in range(16):
                            h_, j_ = divmod(jt, 4)
                            nc.tensor.matmul(pt,
                                             ogTs[(fbi, h_)][:, j_, tt * P: tt * P + tw],
                                             halves[jt // 8][:, jt % 8, :],
                                             start=(jt == 0), stop=(jt == 15))
                        outs = outp.tile([P, 512], f32, tag="outs", name="outs")
                        nc.vector.tensor_copy(outs[:tw, :], pt)
                        nc.sync.dma_start(
                            out=out_d[fbi * L + tt * P: fbi * L + tt * P + tw,
                                      nch * 512:(nch + 1) * 512],
                            in_=outs[:tw, :])

            pend_F = [None]
            decays = {0: emit_decay(0)}
            ogTs = {}

            for c in range(8):
                bi, h = divmod(c, 4)
                eqf, eqb, ekf, ekb = decays.pop(c)
                wcat = wcats.pop(c)
                gw = gws.pop(c)
                if c + 1 < 8:
                    wcats[c + 1] = load_wcat((c + 1) % 4)
                    gws[c + 1] = load_gw((c + 1) % 4)
                owh0 = []
                if h == 3:
                    for hf in range(2):
                        owh = bsl.tile([P, 8, 512], bf16, tag="bslab", name="owh")
                        nc.gpsimd.dma_start(
                            out=owh,
                            in_=ow_d[hf * 8:(hf + 1) * 8, :, 0:512]
                            .rearrange("j p c -> p j c"))
                        owh0.append(owh)

                # ---- q/k projections + decay muls (feature-major [feat, tok]) ----
                qsf = qk.tile([P, 2, L], bf16, tag="qsf", name="qsf")
                qsb = qk.tile([P, 2, L], bf16, tag="qsb", name="qsb")
                ksf = qk.tile([P, 2, L], bf16, tag="ksf", name="ksf")
                ksb = qk.tile([P, 2, L], bf16, tag="ksb", name="ksb")
                for ct in range(2):
                    for o0, w0 in TC2:
                        sl = slice(o0, o0 + w0)
                        qpt = psum(P, 392)
                        for kt in range(8):
                            nc.tensor.matmul(qpt, wcat[:, kt, ct * P:(ct + 1) * P],
                                             xc[:, kt, bi * L + o0: bi * L + o0 + w0],
                                             start=(kt == 0), stop=(kt == 7))
                        nc.vector.tensor_mul(qsf[:, ct, sl], qpt, eqf[:, ct, sl])
                        kpt = psum(P, 392)
                        for kt in range(8):
                            nc.tensor.matmul(kpt, wcat[:, kt, 256 + ct * P: 256 + (ct + 1) * P],
                                             xc[:, kt, bi * L + o0: bi * L + o0 + w0],
                                             start=(kt == 0), stop=(kt == 7))
                        nc.vector.tensor_mul(ksf[:, ct, sl], kpt, ekf[:, ct, sl])
                        nc.vector.tensor_mul(qsb[:, ct, sl], qpt, eqb[:, ct, sl])
                        nc.vector.tensor_mul(ksb[:, ct, sl], kpt, ekb[:, ct, sl])

                if c + 1 < 8:
                    decays[c + 1] = emit_decay(c + 1)
                if pend_F[0] is not None:
                    emit_F(*pend_F[0])
                    pend_F[0] = None

                # ---- A phase dr=0 ----
                am = mid.tile([P, NT7, L], bf16, tag="am", name="am")

                def a_phase(dr, qs, ks, am):
                    for j in range(2):
                        jo, jw = ACH[j]
                        for si in range(NT7):
                            d = si - 4 * j
                            if dr == 0:
                                if si * P > jo + jw - 1:
                                    continue
                                mi_ = None if d < 0 else d
                            else:
                                if si * P + SW[si] - 1 < jo:
                                    continue
                                mi_ = None if d >= 4 else 4 + d
                            sw = SW[si]
                            pt = psum(sw, jw)
                            for ct in range(2):
                                nc.tensor.matmul(pt, ks[:, ct, si * P: si * P + sw],
                                                 qs[:, ct, jo: jo + jw],
                                                 start=(ct == 0), stop=(ct == 1))
                            if mi_ is None:
                                nc.vector.tensor_copy(am[:sw, si, jo: jo + jw], pt)
                            else:
                                nc.vector.tensor_mul(am[:sw, si, jo: jo + jw], pt,
                                                     masks[:sw, mi_, :jw])

                a_phase(0, qsf, ksf, am)

                # ---- v projection (token-major) ----
                vh = mid.tile([P, NT7, HDV], bf16, tag="vh", name="vh")
                for tt in range(NT7):
                    tw = TW[tt]
                    pt = psum(tw, HDV)
                    for kt in range(8):
                        nc.tensor.matmul(pt, xc[:, kt, bi * L + tt * P: bi * L + tt * P + tw],
                                         wcat[:, kt, 512:1024], start=(kt == 0), stop=(kt == 7))
                    nc.vector.tensor_copy(vh[:tw, tt, :], pt)

                # ---- o phase dr=0 ----
                ofn = mid.tile([P, NT7, HDV], bf16, tag="ofn", name="ofn")
                ssq0 = ogp.tile([P, NT7], f32, tag="ssq0", name="ssq0")
                nc.vector.memset(ssq0[:], 0.0)
                scrap = ogp.tile([P, HDV], bf16, tag="lg", name="scrap")
                for tt in range(NT7):
                    tw = TW[tt]
                    pt = psum(tw, HDV)
                    sis = list(range(0, tt + 1))
                    for ii, si in enumerate(sis):
                        nc.tensor.matmul(pt, am[:SW[si], si, tt * P: tt * P + tw],
                                         vh[:SW[si], si, :],
                                         start=(ii == 0), stop=(ii == len(sis) - 1))
                    nc.scalar.activation(scrap[:tw], pt, AF.Square,
                                         accum_out=ssq0[:tw, tt: tt + 1])
                    rsl0 = ogp.tile([P, 1], f32, tag="rsl0", name="rsl0")
                    nc.scalar.activation(rsl0[:tw], ssq0[:tw, tt: tt + 1], AF.Ln,
                                         scale=1.0 / HDV, bias=epst[:tw])
                    nc.scalar.activation(rsl0[:tw], rsl0[:tw], AF.Exp, scale=-0.5)
                    nc.vector.tensor_scalar_mul(ofn[:tw, tt, :], pt, rsl0[:tw])

                # ---- A + o phase dr=1 ----
                am1 = mid.tile([P, NT7, L], bf16, tag="am", name="am1")
                a_phase(1, qsb, ksb, am1)

                o1 = mid.tile([P, NT7, HDV], bf16, tag="o1", name="o1")
                ssq1 = ogp.tile([P, NT7], f32, tag="ssq1", name="ssq1")
                nc.vector.memset(ssq1[:], 0.0)
                for tt in range(NT7):
                    tw = TW[tt]
                    pt = psum(tw, HDV)
                    sis = list(range(tt, NT7))
                    for ii, si in enumerate(sis):
                        nc.tensor.matmul(pt, am1[:SW[si], si, tt * P: tt * P + tw],
                                         vh[:SW[si], si, :],
                                         start=(ii == 0), stop=(ii == len(sis) - 1))
                    nc.scalar.activation(scrap[:tw], pt, AF.Square,
                                         accum_out=ssq1[:tw, tt: tt + 1])
                    nc.vector.tensor_copy(o1[:tw, tt, :], pt)
                rsl1 = ogp.tile([P, NT7], f32, tag="rsl1", name="rsl1")
                nc.scalar.activation(rsl1, ssq1, AF.Ln, scale=1.0 / HDV, bias=epst)
                nc.scalar.activation(rsl1, rsl1, AF.Exp, scale=-0.5)

                # ---- finalize: og = (o1*rsl1 + ofn) * silu(gate), transpose ----
                ogT = ogTp.tile([P, 4, L], bf16, tag="ogT", name="ogT")
                ogTs[(bi, h)] = ogT
                og = mid.tile([P, NT7, HDV], bf16, tag="og", name="og")
                for tt in range(NT7):
                    tw = TW[tt]
                    gpt = psum(tw, HDV)
                    for kt in range(8):
                        nc.tensor.matmul(gpt, xc[:, kt, bi * L + tt * P: bi * L + tt * P + tw],
                                         gw[:, kt, :], start=(kt == 0), stop=(kt == 7))
                    eg = ogp.tile([P, HDV], bf16, tag="eg", name="eg")
                    nc.scalar.activation(eg[:tw], gpt, AF.Exp, scale=-1.0)
                    lg = ogp.tile([P, HDV], bf16, tag="lg", name="lg")
                    nc.scalar.activation(lg[:tw], eg[:tw], AF.Ln, bias=onet[:tw])
                    sg = ogp.tile([P, HDV], bf16, tag="sg", name="sg")
                    nc.scalar.activation(sg[:tw], lg[:tw], AF.Exp, scale=-1.0)
                    gv = ogp.tile([P, HDV], bf16, tag="eg", name="gv")
                    nc.vector.tensor_mul(gv[:tw], gpt, sg[:tw])
                    ob = ogp.tile([P, HDV], bf16, tag="ob", name="ob")
                    nc.vector.scalar_tensor_tensor(ob[:tw], o1[:tw, tt, :],
                                                   rsl1[:tw, tt: tt + 1],
                                                   ofn[:tw, tt, :], ALU.mult, ALU.add)
                    nc.vector.tensor_mul(og[:tw, tt, :], ob[:tw], gv[:tw])
                    tpt = tps.tile([P, 4, P], bf16, tag="tps", name="tpt")
                    for j in range(4):
                        nc.tensor.transpose(tpt[:, j, :tw], og[:tw, tt, j * P:(j + 1) * P],
                                            ident[:tw, :tw])
                    nc.vector.tensor_copy(ogT[:, :, tt * P: tt * P + tw], tpt[:, :, :tw])

                # ---- Stage F: out projection (deferred for bi=0) ----
                if h == 3:
                    if c == 7:
                        emit_F(bi, owh0)
                    else:
                        pend_F[0] = (bi, owh0)

    _legalize_sync_waits(nc)
    return nc


# revision 5
# speedup vs baseline: 1.0304x; 1.0031x over previous
"""Trainium2 Bass kernel for nn_GatedLinearAttention (bidirectional GLA vision block), v2.

Same math as baseline (quadratic masked attention with global decay), restructured
for PE continuity:
  - one activation-table epoch switch total: Silu (conv) -> natural_log_exp for the
    rest: log_sigmoid(u) = -ln(1 + exp(-u)) (Exp then Ln with bias=1), gates via
    silu(g) = g / (1 + exp(-g)) (scalar Exp + DVE reciprocal), rsqrt via Ln+Exp.
  - decay chains computed one combo ahead (scalar/gpsimd/DVE work hidden under PE).
  - cumsum scans + reverse fixes moved to the idle GpSimd engine.
  - og transposed via PE identity-matmul transposes (not serialized DMA transposes).
  - out projection reads transposed og tiles; weights streamed per half-slab.
Data-parallel over batch: 16 batch items -> 8 cores x 2. No collectives.
"""

import os
import sys
from contextlib import ExitStack

for _p in ("/opt/trn_rl_repo", "/root/.axon_site/_ro/trn_rl_repo"):
    if os.path.isdir(_p) and _p not in sys.path:
        sys.path.insert(0, _p)

import numpy as np
import ml_dtypes

import concourse.bass as bass
import concourse.tile as tile
import concourse.mybir as mybir
from concourse.bass_utils import run_bass_kernel_spmd

f32 = mybir.dt.float32
bf16 = mybir.dt.bfloat16
AF = mybir.ActivationFunctionType
ALU = mybir.AluOpType

P = 128
NCORES = 8
B = 2               # batch items per core
L = 784             # tokens per batch item
T = B * L
D = 1024
NH = 4
HDK = 256
HDV = 512
GLN = 16.0
EPS = 1e-5
NT7 = 7
TW = [128, 128, 128, 128, 128, 128, 16]
SW = TW
TC2 = [(0, 392), (392, 392)]
ACH = [(0, 512), (512, 272)]


def _legalize_sync_waits(nc, max_waits=1):
    """Split >1 semaphore waits onto chained NOPs (walrus limitation)."""
    counter = 0
    for fn in nc.m.functions:
        for blk in fn.blocks:
            insts = list(blk.instructions)
            changed = False
            out = []
            for inst in insts:
                si = inst.sync_info
                if si is not None and len(si.on_wait) > max_waits:
                    waits = list(si.on_wait)
                    keep = waits[len(waits) - max_waits:]
                    move = waits[: len(waits) - max_waits]
                    for i in range(0, len(move), max_waits):
                        chunk = move[i: i + max_waits]
                        nop = mybir.InstNoOp(
                            name=f"legalize-wait-nop-{counter}", ins=[], outs=[]
                        )
                        counter += 1
                        nop.engine = inst.engine
                        nop.sync_info = mybir.SyncInfo(on_wait=chunk, on_update=[])
                        out.append(nop)
                    inst.sync_info = mybir.SyncInfo(
                        on_wait=keep, on_update=list(si.on_update)
                    )
                    changed = True
                out.append(inst)
            if changed:
                blk.instructions = out


def _build_program():
    nc = bass.Bass()

    xpad_d = nc.dram_tensor("xpad", [8, P, B * 30 * 30], bf16, kind="ExternalInput")
    cw_d = nc.dram_tensor("cw", [8, P, 9], f32, kind="ExternalInput")
    wcat_d = nc.dram_tensor("wcat", [NH, 8, P, 1024], bf16, kind="ExternalInput")
    gwc_d = nc.dram_tensor("gwc", [NH, 8, P, 512], bf16, kind="ExternalInput")
    gk1w_d = nc.dram_tensor("gk1w", [8, P, 16], bf16, kind="ExternalInput")
    gk2w_d = nc.dram_tensor("gk2w", [16, 2048], bf16, kind="ExternalInput")
    b2n_d = nc.dram_tensor("b2n", [16, P, 1], f32, kind="ExternalInput")
    ow_d = nc.dram_tensor("ow", [16, P, 1024], bf16, kind="ExternalInput")
    f8 = mybir.dt.float8e4
    masks_d = nc.dram_tensor("masks", [8, P, 512], f8, kind="ExternalInput")
    ident_d = nc.dram_tensor("ident", [P, P], bf16, kind="ExternalInput")
    out_d = nc.dram_tensor("out", [T, 1024], f32, kind="ExternalOutput")

    with tile.TileContext(nc) as tc:
        with ExitStack() as ctx:
            cst = ctx.enter_context(tc.tile_pool(name="cst", bufs=1))
            big = ctx.enter_context(tc.tile_pool(name="big", bufs=1))
            wc = ctx.enter_context(tc.tile_pool(name="wc", bufs=2))
            gwp = ctx.enter_context(tc.tile_pool(name="gwp", bufs=1))
            bsl = ctx.enter_context(tc.tile_pool(name="bsl", bufs=2))
            dec = ctx.enter_context(tc.tile_pool(name="dec", bufs=2))
            cpp = ctx.enter_context(tc.tile_pool(name="cpp", bufs=1))
            qk = ctx.enter_context(tc.tile_pool(name="qk", bufs=1))
            mid = ctx.enter_context(tc.tile_pool(name="mid", bufs=1))
            ogp = ctx.enter_context(tc.tile_pool(name="ogp", bufs=1))
            ogTp = ctx.enter_context(tc.tile_pool(name="ogTp", bufs=4))
            sout = ctx.enter_context(tc.tile_pool(name="sout", bufs=2))
            outp = ctx.enter_context(tc.tile_pool(name="outp", bufs=2))
            xpp = ctx.enter_context(tc.tile_pool(name="xpp", bufs=2))
            cdp = ctx.enter_context(tc.tile_pool(name="cdp", bufs=2))
            ps = ctx.enter_context(tc.tile_pool(name="ps", bufs=6, space="PSUM"))
            tps = ctx.enter_context(tc.tile_pool(name="tps", bufs=2, space="PSUM"))

            def psum(rows, cols):
                pstile = ps.tile([P, 512], f32, tag="ps", name="pstile")
                return pstile[:rows, :cols]

            # ---- constants ----
            wvec = cst.tile([P, 8, 9], f32)
            nc.gpsimd.dma_start(out=wvec, in_=cw_d.rearrange("f p t -> p f t"))
            ident = cst.tile([P, P], bf16)
            nc.gpsimd.dma_start(out=ident, in_=ident_d[:])
            zeros = cst.tile([P, 392], bf16)
            nc.vector.memset(zeros[:], 0.0)
            epsone = cst.tile([P, 2], f32)
            nc.vector.memset(epsone[:, 0:1], EPS)
            nc.vector.memset(epsone[:, 1:2], 1.0)
            epst = epsone[:, 0:1]
            onet = epsone[:, 1:2]
            w1 = cst.tile([P, 8, 16], bf16)
            nc.gpsimd.dma_start(out=w1, in_=gk1w_d.rearrange("k p c -> p k c"))

            # ---- persistent activations ----
            xc = big.tile([P, 8, T], bf16)
            gk1o = big.tile([16, T], bf16)

            # ==== Stage A+B per batch item: conv 3x3 + silu (PE diag), gk1 ====
            def stage_ab(bi):
                for ft in range(8):
                    xp = xpp.tile([P, 30, 30], bf16, tag="xp", name="xp")
                    nc.gpsimd.dma_start(
                        out=xp, in_=xpad_d[ft].rearrange("p (b h w) -> p b h w", b=B, h=30)[:, bi])
                    cd = cdp.tile([P, 9, P], bf16, tag="cd", name="cd")
                    for tap in range(9):
                        nc.vector.tensor_scalar_mul(cd[:, tap, :], ident[:],
                                                    wvec[:, ft, tap: tap + 1])
                    pts = [psum(P, 392) for _ in range(2)]
                    for tap in range(9):
                        a, bb = tap // 3, tap % 3
                        for half in range(2):
                            rhs = xp[:, a + half * 14: a + half * 14 + 14, bb: bb + 28]
                            nc.tensor.matmul(pts[half], cd[:, tap, :], rhs,
                                             start=(tap == 0), stop=(tap == 8))
                    for half in range(2):
                        dst = xc[:, ft, bi * L + half * 392: bi * L + (half + 1) * 392]
                        nc.scalar.activation(dst, pts[half], AF.Silu)
                for tc2 in range(2):
                    pt = psum(16, 392)
                    for kt in range(8):
                        nc.tensor.matmul(pt, w1[:, kt, :],
                                         xc[:, kt, bi * L + tc2 * 392: bi * L + (tc2 + 1) * 392],
                                         start=(kt == 0), stop=(kt == 7))
                    nc.vector.tensor_copy(gk1o[:, bi * L + tc2 * 392: bi * L + (tc2 + 1) * 392], pt)

            # ============== decay chains (one combo of lookahead) ==============
            def emit_decay(c):
                bi, h = divmod(c, 4)
                w2 = cpp.tile([16, 4, P], bf16, tag="w2", name="w2")
                nc.gpsimd.dma_start(out=w2[:, 0:2, :],
                                  in_=gk2w_d[:, h * HDK:(h + 1) * HDK].rearrange("k (c p) -> k c p", c=2))
                nc.gpsimd.dma_start(out=w2[:, 2:4, :],
                                  in_=gk2w_d[:, 1024 + h * HDK: 1024 + (h + 1) * HDK].rearrange("k (c p) -> k c p", c=2))
                b2t = dec.tile([P, 4], f32, tag="b2t", name="b2t")
                for mi, mt in enumerate([2 * h, 2 * h + 1, 8 + 2 * h, 8 + 2 * h + 1]):
                    nc.gpsimd.dma_start(out=b2t[:, mi: mi + 1], in_=b2n_d[mt])
                eqf = dec.tile([P, 2, L], bf16, tag="eqf", name="eqf", bufs=1)
                eqb = dec.tile([P, 2, L], bf16, tag="eqb", name="eqb", bufs=1)
                ekf = dec.tile([P, 2, L], bf16, tag="ekf", name="ekf", bufs=1)
                ekb = dec.tile([P, 2, L], bf16, tag="ekb", name="ekb", bufs=1)
                for dr in range(2):
                    et = dec.tile([P, 2, L], bf16, tag="edec", name="et")
                    lp = dec.tile([P, 2, L], bf16, tag="edec", name="lp")
                    cp = cpp.tile([P, 2, L], f32, tag="cp", name="cp")
                    for ct in range(2):
                        mi = dr * 2 + ct
                        for o0, w0 in TC2:
                            upt = psum(P, 392)
                            nc.tensor.matmul(upt, w2[:, mi, :],
                                             gk1o[:, bi * L + o0: bi * L + o0 + w0],
                                             start=True, stop=True)
                            # e = exp(-(u0 + b2)) = exp(-u0 + b2neg)
                            nc.scalar.activation(et[:, ct, o0:o0 + w0], upt, AF.Exp,
                                                 scale=-1.0, bias=b2t[:, mi: mi + 1])
                        # lp = ln(1 + e) = softplus(-u) = -log_sigmoid(u)
                        nc.scalar.activation(lp[:, ct, :], et[:, ct, :], AF.Ln, bias=onet)
                        nc.vector.tensor_tensor_scan(cp[:, ct, 0:392], lp[:, ct, 0:392],
                                                     zeros[:], 0.0, ALU.add, ALU.add)
                        nc.vector.tensor_tensor_scan(cp[:, ct, 392:L], lp[:, ct, 392:L],
                                                     zeros[:], cp[:, ct, 391:392],
                                                     ALU.add, ALU.add)
                        if dr == 1:
                            # cpr = lp - cp + cp_total (reverse-inclusive cumsum)
                            tot = dec.tile([P, 1], f32, tag="tot", name="tot")
                            nc.vector.tensor_copy(tot[:, 0:1], cp[:, ct, L - 1:L])
                            nc.vector.tensor_sub(cp[:, ct, :], lp[:, ct, :], cp[:, ct, :])
                            nc.vector.tensor_scalar_add(cp[:, ct, :], cp[:, ct, :], tot[:, 0:1])
                    dq = eqf if dr == 0 else eqb
                    dk = ekf if dr == 0 else ekb
                    nc.scalar.activation(dq.rearrange("p a b -> p (a b)"),
                                         cp.rearrange("p a b -> p (a b)"),
                                         AF.Exp, scale=-1.0 / GLN)
                    nc.scalar.activation(dk.rearrange("p a b -> p (a b)"),
                                         cp.rearrange("p a b -> p (a b)"),
                                         AF.Exp, scale=1.0 / GLN)
                return eqf, eqb, ekf, ekb

            # prefetch weights for combo 0
            def load_wcat(h):
                w = wc.tile([P, 8, 1024], bf16, tag="wcat", name="wcat")
                nc.gpsimd.dma_start(out=w, in_=wcat_d[h].rearrange("k p c -> p k c"))
                return w

            def load_gw(h):
                g = gwp.tile([P, 8, 512], bf16, tag="gw", name="gw")
                nc.gpsimd.dma_start(out=g, in_=gwc_d[h].rearrange("k p c -> p k c"))
                return g

            stage_ab(0)
            wcats = {0: load_wcat(0)}
            gws = {0: load_gw(0)}
            stage_ab(1)
            masks = cst.tile([P, 8, 512], f8)
            nc.gpsimd.dma_start(out=masks, in_=masks_d.rearrange("m p t -> p m t"))

            def emit_F(fbi, owh0):
                for nch in range(2):
                    if nch == 0:
                        halves = owh0
                    else:
                        halves = []
                        for hf in range(2):
                            owh = bsl.tile([P, 8, 512], bf16, tag="bslab", name="owh")
                            nc.gpsimd.dma_start(
                                out=owh,
                                in_=ow_d[hf * 8:(hf + 1) * 8, :, 512:1024]
                                .rearrange("j p c -> p j c"))
                            halves.append(owh)
                    for tt in range(NT7):
                        tw = TW[tt]
                        pt = psum(tw, 512)
                        for jt in range(16):
                            h_, j_ = divmod(jt, 4)
                            nc.tensor.matmul(pt,
                                             ogTs[(fbi, h_)][:, j_, tt * P: tt * P + tw],
                                             halves[jt // 8][:, jt % 8, :],
                                             start=(jt == 0), stop=(jt == 15))
                        outs = outp.tile([P, 512], f32, tag="outs", name="outs")
                        nc.vector.tensor_copy(outs[:tw, :], pt)
                        nc.sync.dma_start(
                            out=out_d[fbi * L + tt * P: fbi * L + tt * P + tw,
                                      nch * 512:(nch + 1) * 512],
                            in_=outs[:tw, :])

            pend_F = [None]
            decays = {0: emit_decay(0)}
            ogTs = {}

            for c in range(8):
                bi, h = divmod(c, 4)
                eqf, eqb, ekf, ekb = decays.pop(c)
                wcat = wcats.pop(c)
                gw = gws.pop(c)
                if c + 1 < 8:
                    wcats[c + 1] = load_wcat((c + 1) % 4)
                    gws[c + 1] = load_gw((c + 1) % 4)
                owh0 = []
                if h == 3:
                    for hf in range(2):
                        owh = bsl.tile([P, 8, 512], bf16, tag="bslab", name="owh")
                        nc.gpsimd.dma_start(
                            out=owh,
                            in_=ow_d[hf * 8:(hf + 1) * 8, :, 0:512]
                            .rearrange("j p c -> p j c"))
                        owh0.append(owh)

                # ---- q/k projections + decay muls (feature-major [feat, tok]) ----
                qsf = qk.tile([P, 2, L], bf16, tag="qsf", name="qsf")
                qsb = qk.tile([P, 2, L], bf16, tag="qsb", name="qsb")
                ksf = qk.tile([P, 2, L], bf16, tag="ksf", name="ksf")
                ksb = qk.tile([P, 2, L], bf16, tag="ksb", name="ksb")
                for ct in range(2):
                    for o0, w0 in TC2:
                        sl = slice(o0, o0 + w0)
                        qpt = psum(P, 392)
                        for kt in range(8):
                            nc.tensor.matmul(qpt, wcat[:, kt, ct * P:(ct + 1) * P],
                                             xc[:, kt, bi * L + o0: bi * L + o0 + w0],
                                             start=(kt == 0), stop=(kt == 7))
                        nc.vector.tensor_mul(qsf[:, ct, sl], qpt, eqf[:, ct, sl])
                        kpt = psum(P, 392)
                        for kt in range(8):
                            nc.tensor.matmul(kpt, wcat[:, kt, 256 + ct * P: 256 + (ct + 1) * P],
                                             xc[:, kt, bi * L + o0: bi * L + o0 + w0],
                                             start=(kt == 0), stop=(kt == 7))
                        nc.vector.tensor_mul(ksf[:, ct, sl], kpt, ekf[:, ct, sl])
                        nc.vector.tensor_mul(qsb[:, ct, sl], qpt, eqb[:, ct, sl])
                        nc.vector.tensor_mul(ksb[:, ct, sl], kpt, ekb[:, ct, sl])

                if c + 1 < 8:
                    decays[c + 1] = emit_decay(c + 1)
                if pend_F[0] is not None:
                    emit_F(*pend_F[0])
                    pend_F[0] = None

                # ---- A phase dr=0 ----
                am = mid.tile([P, NT7, L], bf16, tag="am", name="am")

                def a_phase(dr, qs, ks, am):
                    for j in range(2):
                        jo, jw = ACH[j]
                        for si in range(NT7):
                            d = si - 4 * j
                            if dr == 0:
                                if si * P > jo + jw - 1:
                                    continue
                                mi_ = None if d < 0 else d
                            else:
                                if si * P + SW[si] - 1 < jo:
                                    continue
                                mi_ = None if d >= 4 else 4 + d
                            sw = SW[si]
                            pt = psum(sw, jw)
                            for ct in range(2):
                                nc.tensor.matmul(pt, ks[:, ct, si * P: si * P + sw],
                                                 qs[:, ct, jo: jo + jw],
                                                 start=(ct == 0), stop=(ct == 1))
                            if mi_ is None:
                                nc.vector.tensor_copy(am[:sw, si, jo: jo + jw], pt)
                            else:
                                nc.vector.tensor_mul(am[:sw, si, jo: jo + jw], pt,
                                                     masks[:sw, mi_, :jw])

                a_phase(0, qsf, ksf, am)

                # ---- v projection (token-major) ----
                vh = mid.tile([P, NT7, HDV], bf16, tag="vh", name="vh")
                for tt in range(NT7):
                    tw = TW[tt]
                    pt = psum(tw, HDV)
                    for kt in range(8):
                        nc.tensor.matmul(pt, xc[:, kt, bi * L + tt * P: bi * L + tt * P + tw],
                                         wcat[:, kt, 512:1024], start=(kt == 0), stop=(kt == 7))
                    nc.vector.tensor_copy(vh[:tw, tt, :], pt)

                # ---- o phase dr=0 ----
                ofn = mid.tile([P, NT7, HDV], bf16, tag="ofn", name="ofn")
                ssq0 = ogp.tile([P, NT7], f32, tag="ssq0", name="ssq0")
                nc.vector.memset(ssq0[:], 0.0)
                scrap = ogp.tile([P, HDV], bf16, tag="lg", name="scrap")
                for tt in range(NT7):
                    tw = TW[tt]
                    pt = psum(tw, HDV)
                    sis = list(range(0, tt + 1))
                    for ii, si in enumerate(sis):
                        nc.tensor.matmul(pt, am[:SW[si], si, tt * P: tt * P + tw],
                                         vh[:SW[si], si, :],
                                         start=(ii == 0), stop=(ii == len(sis) - 1))
                    nc.scalar.activation(scrap[:tw], pt, AF.Square,
                                         accum_out=ssq0[:tw, tt: tt + 1])
                    rsl0 = ogp.tile([P, 1], f32, tag="rsl0", name="rsl0")
                    nc.scalar.activation(rsl0[:tw], ssq0[:tw, tt: tt + 1], AF.Ln,
                                         scale=1.0 / HDV, bias=epst[:tw])
                    nc.scalar.activation(rsl0[:tw], rsl0[:tw], AF.Exp, scale=-0.5)
                    nc.vector.tensor_scalar_mul(ofn[:tw, tt, :], pt, rsl0[:tw])

                # ---- A + o phase dr=1 ----
                am1 = mid.tile([P, NT7, L], bf16, tag="am", name="am1")
                a_phase(1, qsb, ksb, am1)

                o1 = mid.tile([P, NT7, HDV], bf16, tag="o1", name="o1")
                ssq1 = ogp.tile([P, NT7], f32, tag="ssq1", name="ssq1")
                nc.vector.memset(ssq1[:], 0.0)
                for tt in range(NT7):
                    tw = TW[tt]
                    pt = psum(tw, HDV)
                    sis = list(range(tt, NT7))
                    for ii, si in enumerate(sis):
                        nc.tensor.matmul(pt, am1[:SW[si], si, tt * P: tt * P + tw],
                                         vh[:SW[si], si, :],
                                         start=(ii == 0), stop=(ii == len(sis) - 1))
                    nc.scalar.activation(scrap[:tw], pt, AF.Square,
                                         accum_out=ssq1[:tw, tt: tt + 1])
                    nc.vector.tensor_copy(o1[:tw, tt, :], pt)
                rsl1 = ogp.tile([P, NT7], f32, tag="rsl1", name="rsl1")
                nc.scalar.activation(rsl1, ssq1, AF.Ln, scale=1.0 / HDV, bias=epst)
                nc.scalar.activation(rsl1, rsl1, AF.Exp, scale=-0.5)

                # ---- finalize: og = (o1*rsl1 + ofn) * silu(gate), transpose ----
                ogT = ogTp.tile([P, 4, L], bf16, tag="ogT", name="ogT")
                ogTs[(bi, h)] = ogT
                og = mid.tile([P, NT7, HDV], bf16, tag="og", name="og")
                for tt in range(NT7):
                    tw = TW[tt]
                    gpt = psum(tw, HDV)
                    for kt in range(8):
                        nc.tensor.matmul(gpt, xc[:, kt, bi * L + tt * P: bi * L + tt * P + tw],
                                         gw[:, kt, :], start=(kt == 0), stop=(kt == 7))
                    eg = ogp.tile([P, HDV], bf16, tag="eg", name="eg")
                    nc.scalar.activation(eg[:tw], gpt, AF.Exp, scale=-1.0)
                    lg = ogp.tile([P, HDV], bf16, tag="lg", name="lg")
                    nc.scalar.activation(lg[:tw], eg[:tw], AF.Ln, bias=onet[:tw])
                    sg = ogp.tile([P, HDV], bf16, tag="sg", name="sg")
                    nc.scalar.activation(sg[:tw], lg[:tw], AF.Exp, scale=-1.0)
                    gv = ogp.tile([P, HDV], bf16, tag="eg", name="gv")
                    nc.vector.tensor_mul(gv[:tw], gpt, sg[:tw])
                    ob = ogp.tile([P, HDV], bf16, tag="ob", name="ob")
                    nc.vector.scalar_tensor_tensor(ob[:tw], o1[:tw, tt, :],
                                                   rsl1[:tw, tt: tt + 1],
                                                   ofn[:tw, tt, :], ALU.mult, ALU.add)
                    nc.vector.tensor_mul(og[:tw, tt, :], ob[:tw], gv[:tw])
                    if h == 3:
                        tpt = tps.tile([P, 4, P], bf16, tag="tps", name="tpt")
                        for j in range(4):
                            nc.tensor.transpose(tpt[:, j, :tw], og[:tw, tt, j * P:(j + 1) * P],
                                                ident[:tw, :tw])
                        nc.vector.tensor_copy(ogT[:, :, tt * P: tt * P + tw], tpt[:, :, :tw])
                    else:
                        for j in range(4):
                            nc.sync.dma_start_transpose(ogT[:, j, tt * P: tt * P + tw],
                                                        og[:tw, tt, j * P:(j + 1) * P])

                # ---- Stage F: out projection (deferred for bi=0) ----
                if h == 3:
                    if c == 7:
                        emit_F(bi, owh0)
                    else:
                        pend_F[0] = (bi, owh0)

    _legalize_sync_waits(nc)
    return nc
